# revision 26
# baseline (speedup 1.0000x reference)
"""GQA attention with LoRA-Q, tensor-parallel over 8 TRN2 cores.

Sharding (per core i of 8):
  - Q heads 4i..4i+3 (256 q-dims) and KV head i (GQA: repeat_interleave maps
    q heads [4i,4i+4) exactly onto kv head i).
  - Wq (with LoRA folded: Wq_eff = Wq + lora_B @ lora_A), Wk, Wv row-sharded;
    Wo row-parallel on its input (head) dim: each core computes the full-width
    partial y^T from its own 256 head-dims; four per-T-chunk ReduceScatter(add)
    ops (overlapped with attention of later chunks) leave each core its
    256-feature slice of y^T (transposed + concatenated on the host).

Structure is a single fused c-outer pipeline over the four 512-token chunks:
  proj c -> rope c -> attention (head pairs, shared kv) c -> norm c ->
  partial-Wo drain c -> ReduceScatter #c
so the collective and DMA traffic hide under attention of later chunks.

Precision: the QKV/V projections run as fp8e4 DoubleRow (256-deep contraction
per pass) in THREE split-precision terms — x and the (64x prescaled) weights
are each decomposed on the host into hi + lo fp8 parts and the x_lo*w_lo term
is dropped: xh@wh + xh@wl + xl@wh costs 3/4 of the bf16 chain at ~2e-3
relative error (better than bf16). Plain fp8 everywhere was measured at 2-5%
error per matmul family (relative error of a dot product equals the
per-element quantization error; it does not average down), which blows the
2e-2 budget, so scores/PV/Wo stay bf16. Softmax without max subtraction
(scores are bounded: |S/8| <= ~7), denominator fused into the PV matmul via
an appended ones-column on V. RoPE's rotate-half is a signed permutation
matmul on PE (no partition-shuffle DMAs). Score (ST) matmuls and the exp are
column-trimmed on the block-diagonal to skip fully-masked regions.
"""

import numpy as np
import ml_dtypes

import concourse.bass as bass
import concourse.mybir as mybir
import concourse.tile as tile
from concourse import bacc
from concourse.bass_utils import run_bass_kernel_spmd

BF16 = mybir.dt.bfloat16
F32 = mybir.dt.float32
FP8 = mybir.dt.float8e4
DR = mybir.MatmulPerfMode.DoubleRow

N_CORES = 8
T = 2048
D = 2048
HD = 64          # head dim
NH = 32          # total q heads
NKV = 8          # total kv heads
NH_LOC = NH // N_CORES       # 4 q heads per core
QW = NH_LOC * HD             # 256 q dims per core
P = 128
KT = D // P                  # 16 contraction tiles
KT2 = KT // 2                # 8 DoubleRow pairs
CH = 512                     # T-chunk
NCH = T // CH                # 4 chunks
NJ = T // P                  # 16 k-blocks
SCALE = 1.0 / 8.0            # 1/sqrt(64)
WS = 64.0                    # host-side weight scale (fp8 subnormal dodge)
IWS = 1.0 / WS


def build_bass():
    nc = bacc.Bacc(None, num_devices=N_CORES)

    # I/O
    xh_d = nc.dram_tensor("x8h", [D, T], FP8, kind="ExternalInput")
    xl_d = nc.dram_tensor("x8l", [D, T], FP8, kind="ExternalInput")
    wh_d = nc.dram_tensor("w8h", [D, QW + 2 * HD], FP8, kind="ExternalInput")
    wl_d = nc.dram_tensor("w8l", [D, QW + 2 * HD], FP8, kind="ExternalInput")
    woT_d = nc.dram_tensor("woT", [QW, D], BF16, kind="ExternalInput")
    woh_d = nc.dram_tensor("wo8h", [QW, D], FP8, kind="ExternalInput")
    wol_d = nc.dram_tensor("wo8l", [QW, D], FP8, kind="ExternalInput")
    cos2_d = nc.dram_tensor("cos2", [P, T], BF16, kind="ExternalInput")
    sin2_d = nc.dram_tensor("sin2", [P, T], BF16, kind="ExternalInput")
    tri_d = nc.dram_tensor("tri", [P, P], BF16, kind="ExternalInput")
    perm_d = nc.dram_tensor("perm", [P, P], BF16, kind="ExternalInput")
    y_d = nc.dram_tensor("y", [NCH, QW, CH], BF16, kind="ExternalOutput")

    with tile.TileContext(nc, num_cores=N_CORES) as tc:
        _body(nc, tc, xh_d, xl_d, wh_d, wl_d, woT_d, woh_d, wol_d, cos2_d,
              sin2_d, tri_d, perm_d, y_d)
    nc.compile()
    return nc


def _body(nc, tc, xh_d, xl_d, wh_d, wl_d, woT_d, woh_d, wol_d, cos2_d,
          sin2_d, tri_d, perm_d, y_d):
    import contextlib
    import itertools

    ctx = contextlib.ExitStack()
    with ctx:
        consts = ctx.enter_context(tc.tile_pool(name="consts", bufs=1))
        big = ctx.enter_context(tc.tile_pool(name="big", bufs=1))
        work = ctx.enter_context(tc.tile_pool(name="work", bufs=1))
        rp = ctx.enter_context(tc.tile_pool(name="rp", bufs=3))
        pt_p = ctx.enter_context(tc.tile_pool(name="pt_p", bufs=6))
        nrm = ctx.enter_context(tc.tile_pool(name="nrm", bufs=2))
        pst = ctx.enter_context(tc.tile_pool(name="pst", bufs=2))
        stP = ctx.enter_context(tc.tile_pool(name="stP", bufs=2, space="PSUM"))
        otP = ctx.enter_context(tc.tile_pool(name="otP", bufs=2, space="PSUM"))
        drP = ctx.enter_context(tc.tile_pool(name="drP", bufs=2, space="PSUM"))
        dram = ctx.enter_context(tc.tile_pool(name="dram", bufs=1, space="DRAM"))

        # ---- constant tiles (few big DMAs; the DMA path serializes per
        # dma_start, so count matters more than size)
        wh_sb = consts.tile([P, KT, QW + 2 * HD], FP8)
        wl_sb = consts.tile([P, KT, QW + 2 * HD], FP8)
        wh_r = wh_d.rearrange("(kt p) m -> p kt m", p=P)
        wl_r = wl_d.rearrange("(kt p) m -> p kt m", p=P)
        xh_sb = big.tile([P, KT, T], FP8, tag="bigh", name="xh_sb")
        xl_sb = big.tile([P, KT, T], FP8, tag="bigl", name="xl_sb")
        xh_r = xh_d.rearrange("(kt p) t -> p kt t", p=P)
        xl_r = xl_d.rearrange("(kt p) t -> p kt t", p=P)
        MK = 2 * P  # m=2 (k/v) weight columns start here; loaded first
        nc.sync.dma_start(wh_sb[:, :, MK:], wh_r[:, :, MK:])
        nc.sync.dma_start(xh_sb[:, :, 0:CH], xh_r[:, :, 0:CH])
        nc.sync.dma_start(wl_sb[:, :, MK:], wl_r[:, :, MK:])
        nc.sync.dma_start(xl_sb[:, :, 0:CH], xl_r[:, :, 0:CH])
        perm_sb = consts.tile([P, P], BF16)
        nc.sync.dma_start(perm_sb, perm_d[:])
        tri_sb = consts.tile([P, P], BF16)
        nc.sync.dma_start(tri_sb, tri_d[:])
        cos2_sb = consts.tile([P, T], BF16)
        nc.sync.dma_start(cos2_sb, cos2_d[:])
        sin2_sb = consts.tile([P, T], BF16)
        nc.sync.dma_start(sin2_sb, sin2_d[:])
        nc.sync.dma_start(wh_sb[:, :, 0:MK], wh_r[:, :, 0:MK])
        nc.sync.dma_start(wl_sb[:, :, 0:MK], wl_r[:, :, 0:MK])
        nc.sync.dma_start(xh_sb[:, :, CH : 2 * CH], xh_r[:, :, CH : 2 * CH])
        nc.sync.dma_start(xl_sb[:, :, CH : 2 * CH], xl_r[:, :, CH : 2 * CH])
        woT2_sb = consts.tile([P, 2, D], BF16)
        woT2_r = woT_d.rearrange("(g p) o -> p g o", p=P)
        woh_sb = consts.tile([P, 2, D], FP8)
        wol_sb = consts.tile([P, 2, D], FP8)
        nc.sync.dma_start(woh_sb, woh_d.rearrange("(g p) o -> p g o", p=P)[:, :, :])
        nc.sync.dma_start(wol_sb, wol_d.rearrange("(g p) o -> p g o", p=P)[:, :, :])

        ones64 = consts.tile([1, HD], BF16)
        nc.vector.memset(ones64, 1.0)

        # v with ones column appended: [tk(P), j, HD+1]
        v_aug = work.tile([P, NJ, HD + 1], BF16)
        nc.vector.memset(v_aug[:, :, HD : HD + 1], 1.0)

        # k^T duplicated on both partition halves so odd heads (whose rope
        # output lives at partitions 64:128) can matmul base-aligned
        kT2 = work.tile([P, T], BF16)
        # rope'd q pairs for all chunks: qAll[64*(h%2)+d, h//2, t]
        qAll = work.tile([P, 2, T], BF16)
        # O^T repacked to 128 partitions: OT128[64*(h%2)+d, h//2, t]
        # (partition p of pair g is local head-dim 128*g+p, matching woT2)
        OT128 = work.tile([P, 2, T], BF16)
        # final chunk's O in fp8 for the DoubleRow drain (the O-quantization
        # error only touches the last quarter of tokens: ~1.4%% overall)
        OT8 = work.tile([P, 2, CH], FP8)

        # per-chunk DRAM tiles: a shared tile would make chunk c+1's partT
        # writes falsely wait on ReduceScatter c's reads (coarse DRAM deps)
        partT_c = [
            dram.tile([D, CH], BF16, name=f"partT{c}", tag=f"partT{c}")
            for c in range(NCH)
        ]
        partT_rc = [t.rearrange("(ot p) t -> p ot t", p=P) for t in partT_c]
        ysT_c = [
            dram.tile([QW, CH], BF16, name=f"ysT{c}", tag=f"ysT{c}")
            for c in range(NCH)
        ]

        TERMS = None  # bound below; (w, x) split-precision term pairs

        def proj3_term(ps, wcols, rhs_sl, t, k2, rev=False):
            """One DoubleRow pair of split-precision term t (term-major so
            the hi*hi sweep starts after only the hi DMAs have landed)."""
            w, x = ((wh_sb, xh_sb), (wl_sb, xh_sb), (wh_sb, xl_sb))[t]
            if rev:
                lhsT = x[:, 2 * k2 : 2 * k2 + 2, rhs_sl]
                rhs = w[:, 2 * k2 : 2 * k2 + 2, wcols]
            else:
                lhsT = w[:, 2 * k2 : 2 * k2 + 2, wcols]
                rhs = x[:, 2 * k2 : 2 * k2 + 2, rhs_sl]
            nc.tensor.matmul(
                ps,
                lhsT=lhsT,
                rhs=rhs,
                start=(k2 == 0 and t == 0),
                stop=(k2 == KT2 - 1 and t == 2),
                perf_mode=DR,
            )

        def emit_proj_rope(c):
            """QKV projection + RoPE + t-partitioned V for chunk c, as a
            generator so chunks >= 1 can ride attention's PE bubbles."""
            sl = slice(c * CH, (c + 1) * CH)
            projT = rp.tile([P, 3, CH], BF16, tag="projT")
            for m in (2, 0, 1):
                ps = drP.tile([P, CH], F32, tag="dr")
                nmm = 0
                for t in range(3):
                    for k2 in range(KT2):
                        proj3_term(ps, slice(m * P, (m + 1) * P), sl, t, k2)
                        nmm += 1
                        if nmm % 8 == 7:
                            yield
                nc.vector.tensor_scalar_mul(projT[:, m, :], ps, IWS)
                yield
                if m == 2:
                    # RoPE k -> kT2[0:64, sl] + dup to [64:128]
                    ksh = drP.tile([P, CH], F32, tag="dr")
                    nc.tensor.matmul(
                        ksh[0:HD, :], lhsT=perm_sb[0:HD, 0:HD],
                        rhs=projT[0:HD, 2, :], start=True, stop=True,
                    )
                    kt2 = rp.tile([P, CH], BF16, tag="t2")
                    nc.vector.tensor_mul(kt2[0:HD, :], ksh[0:HD, :], sin2_sb[0:HD, sl])
                    veng = nc.vector if c == 0 else nc.gpsimd
                    kt1 = rp.tile([P, CH], BF16, tag="t1")
                    veng.tensor_mul(kt1[0:HD, :], projT[0:HD, 2, :], cos2_sb[0:HD, sl])
                    veng.tensor_add(kT2[0:HD, sl], kt1[0:HD, :], kt2[0:HD, :])
                    nc.gpsimd.dma_start(kT2[HD:P, sl], kT2[0:HD, sl])
                    if c == 1:
                        nc.sync.dma_start(woT2_sb, woT2_r[:, :, :])
                    if 1 <= c <= 2:
                        nx = slice((c + 1) * CH, (c + 2) * CH)
                        nc.sync.dma_start(xh_sb[:, :, nx], xh_r[:, :, nx])
                        nc.sync.dma_start(xl_sb[:, :, nx], xl_r[:, :, nx])
                    yield
                    # v in t-partitioned layout (tiny free dim: cheap matmuls)
                    for tb in range(4):
                        j = 4 * c + tb
                        pv = drP.tile([P, CH], F32, tag="dr")
                        for t in range(3):
                            for k2 in range(KT2):
                                proj3_term(
                                    pv[:, 0:HD],
                                    slice(QW + HD, QW + 2 * HD),
                                    slice(j * P, (j + 1) * P),
                                    t, k2, rev=True,
                                )
                        nc.vector.tensor_scalar_mul(
                            v_aug[:, j, 0:HD], pv[:, 0:HD], IWS
                        )
                        yield
                else:
                    s = m
                    qsh = drP.tile([P, CH], F32, tag="dr")
                    nc.tensor.matmul(
                        qsh, lhsT=perm_sb, rhs=projT[:, s, :],
                        start=True, stop=True,
                    )
                    yield
                    t2 = rp.tile([P, CH], BF16, tag="t2")
                    nc.vector.tensor_mul(t2, qsh, sin2_sb[:, sl])
                    veng = nc.vector if c == 0 else nc.gpsimd
                    t1 = rp.tile([P, CH], BF16, tag="t1")
                    veng.tensor_mul(t1, projT[:, s, :], cos2_sb[:, sl])
                    veng.tensor_add(qAll[:, s, sl], t1, t2)
                    yield

        def drain_gen(c, final=False):
            """Chunk c's partial-Wo drain + partT DMAs + ReduceScatter,
            interleaved into attention (c+1)'s j-loop as PE filler. The final
            drain feeds the tail ReduceScatter, so it balances the PSUM-drain
            copies across ACT+DVE and ships partT in small early pieces to
            get the collective started as soon as possible."""
            sl = slice(c * CH, (c + 1) * CH)
            fast = final or c == NCH - 2
            grp = 4
            pstage = pst.tile([P, KT, CH], BF16, tag="pst")
            for o_t in range(KT):
                # the final drain alternates between the drain and (by then
                # idle) attention PSUM pools: 4 banks of runway keep the
                # PSUM->SBUF copies fed so the scheduler never parks the
                # end-of-program drain barrier in front of them
                if final and o_t % 2 == 1:
                    ps = otP.tile([P, CH], F32, tag="ot")
                else:
                    ps = drP.tile([P, CH], F32, tag="dr")
                if final:
                    for t, w in enumerate((woh_sb, wol_sb)):
                        nc.tensor.matmul(
                            ps,
                            lhsT=w[:, :, o_t * P : (o_t + 1) * P],
                            rhs=OT8,
                            start=(t == 0),
                            stop=(t == 1),
                            perf_mode=DR,
                        )
                else:
                    for g in range(2):
                        nc.tensor.matmul(
                            ps,
                            lhsT=woT2_sb[:, g, o_t * P : (o_t + 1) * P],
                            rhs=OT128[:, g, sl],
                            start=(g == 0),
                            stop=(g == 1),
                        )
                yield
                if fast:
                    # late chunks feed the tail ReduceScatter chain: copies
                    # prefer DVE (ACT is exp-bound), with partT pieces issued
                    # from the gpsimd queue right after each odd copy so the
                    # collective starts the moment the drain lands.
                    if final and o_t % 2 == 0:
                        nc.scalar.mul(pstage[:, o_t, :], ps, IWS if final else 1.0)
                    elif final:
                        nc.vector.tensor_scalar_mul(pstage[:, o_t, :], ps, IWS)
                    else:
                        nc.vector.tensor_copy(pstage[:, o_t, :], ps)
                    if o_t % 4 == 3:
                        nc.gpsimd.dma_start(
                            partT_rc[c][:, o_t - 3 : o_t + 1, :],
                            pstage[:, o_t - 3 : o_t + 1, :],
                        )
                else:
                    if o_t % 4 == 0:
                        nc.scalar.copy(pstage[:, o_t, :], ps)
                    else:
                        nc.vector.tensor_copy(pstage[:, o_t, :], ps)
                    if o_t % grp == grp - 1:
                        hb = o_t - grp + 1
                        nc.sync.dma_start(
                            partT_rc[c][:, hb : o_t + 1, :],
                            pstage[:, hb : o_t + 1, :],
                        )
                yield
            emit_rs(c)

        def emit_rs(c):
            """ReduceScatter chunk c. The y output DMAs are all deferred to
            the end of the program: a y DMA waits ~21us on its collective,
            and on the in-order SP queue that wait would head-block the next
            chunk's partT DMAs, serializing every collective."""
            nc.gpsimd.collective_compute(
                "ReduceScatter",
                mybir.AluOpType.add,
                replica_groups=[list(range(N_CORES))],
                ins=[partT_c[c][:, :]],
                outs=[ysT_c[c][:, :]],
            )

        def emit_attention(c, filler, norm_now=False):
            """Attention for the 4 local heads of q-chunk c, exp grouped over
            2 adjacent k-blocks. Block-diagonal pairs are column-trimmed (the
            first 256*dg columns are fully masked). Pulls from filler between
            ST and PV so PE never idles while Exp runs."""
            nj2 = 2 * c + 2
            stg = nrm.tile([HD + 1, NH_LOC, CH], BF16, tag="stg")
            horder = (1, 3, 0, 2) if norm_now else (0, 2, 1, 3)
            for hpos, h in enumerate(horder):
                lo = HD * (h % 2)
                qrhs = qAll[lo : lo + HD, h // 2, c * CH : (c + 1) * CH]
                ot = otP.tile([P, CH], F32, tag="ot")

                def do_st(j2):
                    x0 = max(0, 2 * P * (j2 - 2 * c))
                    st = stP.tile([P, 2, CH], F32, tag="st")
                    for i in range(2):
                        nc.tensor.matmul(
                            st[:, i, x0:CH],
                            lhsT=kT2[lo : lo + HD, (2 * j2 + i) * P : (2 * j2 + i + 1) * P],
                            rhs=qrhs[:, x0:CH],
                            start=True,
                            stop=True,
                        )
                    return st

                EXPA = 128.0 * 1.4426950409 / 8.0   # bf16-bit exp slope
                EXPB = 16250.5                       # bias 16256 - sigma

                def do_rest(st, j2):
                    x0 = max(0, 2 * P * (j2 - 2 * c))
                    pt = pt_p.tile([P, 2, CH], BF16, tag="pt")
                    if False and c == NCH - 1 and hpos < 2 and j2 < 2 * c:
                        # ACT is the bottleneck of the last chunk: route a
                        # third of its off-diagonal exps to DVE as a
                        # Schraudolph bit-trick (synthesize the bf16 bit
                        # pattern of e^(s/8) with one mult+add, ~2% rms on
                        # ~12% of the softmax mass)
                        nc.vector.tensor_scalar(
                            pt.bitcast(mybir.dt.int16), st[:, :, :],
                            EXPA, EXPB,
                            mybir.AluOpType.mult, mybir.AluOpType.add,
                        )
                    else:
                        nc.scalar.activation(
                            pt[:, :, x0:CH], st[:, :, x0:CH],
                            mybir.ActivationFunctionType.Exp, scale=SCALE,
                        )
                    if j2 >= 2 * c:
                        # diagonal pair: per-block column trim. Block at diag
                        # offset d contributes nothing to cols < 128d; only
                        # the 128-wide triangle at [128d, 128d+128) needs
                        # masking; cols >= 128(d+1) are fully valid.
                        for i in range(2):
                            j = 2 * j2 + i
                            d = j - 4 * c
                            nc.vector.tensor_mul(
                                pt[:, i, P * d : P * (d + 1)],
                                pt[:, i, P * d : P * (d + 1)],
                                tri_sb,
                            )
                            nc.tensor.matmul(
                                ot[0 : HD + 1, P * d : CH],
                                lhsT=v_aug[:, j, :],
                                rhs=pt[:, i, P * d : CH],
                                start=(j == 0),
                                stop=(j == 4 * c + 3),
                                skip_group_check=True,
                            )
                    else:
                        for i in range(2):
                            j = 2 * j2 + i
                            nc.tensor.matmul(
                                ot[0 : HD + 1, :],
                                lhsT=v_aug[:, j, :],
                                rhs=pt[:, i, :],
                                start=(j == 0),
                                stop=False,
                                skip_group_check=True,
                            )

                st_cur = do_st(0)
                npull = 5 if c >= NCH - 2 else 3
                for j2 in range(nj2):
                    st_next = do_st(j2 + 1) if j2 + 1 < nj2 else None
                    next(filler, None)
                    do_rest(st_cur, j2)
                    for _ in range(npull - 1):
                        next(filler, None)
                    st_cur = st_next

                # stage unnormalized O^T + denominator row (bf16)
                if h % 2 == 0:
                    nc.scalar.copy(stg[:, h, :], ot[0 : HD + 1, :])
                else:
                    nc.vector.tensor_copy(stg[:, h, :], ot[0 : HD + 1, :])
                if norm_now:
                    # final chunk: normalize this head immediately so the
                    # partial-Wo drain can start right at attention end
                    for _ in norm_head(c, stg, h):
                        pass
            return stg

        def norm_head(c, stg, h):
            """Softmax normalization for one head of chunk c."""
            sl = slice(c * CH, (c + 1) * CH)
            rcpT = nrm.tile([1, NH_LOC, CH], BF16, tag="rcp")
            with nc.allow_low_precision("softmax denom in bf16 is fine"):
                nc.vector.reciprocal(
                    rcpT[0:1, h, :], stg[HD : HD + 1, h, :]
                )
            bc = otP.tile([P, CH], F32, tag="ot")
            nc.tensor.matmul(
                bc[0:HD, :], lhsT=ones64, rhs=rcpT[0:1, h, :],
                start=True, stop=True,
            )
            if c == NCH - 1:
                if h % 2 == 0:
                    nc.vector.tensor_mul(
                        OT8[0:HD, h // 2, :], stg[0:HD, h, :], bc[0:HD, :]
                    )
                else:
                    odd8 = nrm.tile([HD, CH], FP8, tag="odd8")
                    nc.vector.tensor_mul(odd8, stg[0:HD, h, :], bc[0:HD, :])
                    nc.gpsimd.dma_start(OT8[HD:P, h // 2, :], odd8)
            elif h % 2 == 0:
                nc.vector.tensor_mul(
                    OT128[0:HD, h // 2, sl], stg[0:HD, h, :], bc[0:HD, :]
                )
            else:
                oddt = nrm.tile([HD, CH], BF16, tag="oddt")
                nc.vector.tensor_mul(oddt, stg[0:HD, h, :], bc[0:HD, :])
                nc.gpsimd.dma_start(OT128[HD:P, h // 2, sl], oddt)
            yield

        def emit_norm(c, stg):
            """Softmax normalization for chunk c (4 heads)."""
            for h in range(NH_LOC):
                yield from norm_head(c, stg, h)

        # ---- prologue: chunk 0's proj/rope dense; everything else (proj of
        # later chunks, norm + partial-Wo drain + ReduceScatter of earlier
        # chunks) rides attention's PE bubbles via the filler generators.
        for _ in emit_proj_rope(0):
            pass
        filler = emit_proj_rope(1)
        for c in range(NCH):
            late = c >= NCH - 2
            stg = emit_attention(c, filler, norm_now=late)
            for _ in filler:
                pass
            gens = []
            if c + 2 < NCH:
                gens.append(emit_proj_rope(c + 2))
            if not late:
                gens.append(emit_norm(c, stg))
            gens.append(drain_gen(c, final=(c == NCH - 1)))
            filler = itertools.chain(*gens)
        # final norm+drain runs dense (its ReduceScatter is emitted inline)
        for _ in filler:
            pass
        y_r = y_d.rearrange("c (r p) t -> p c r t", p=P)
        for c in range(NCH):
            ys_r = ysT_c[c].rearrange("(r p) t -> p r t", p=P)
            nc.sync.dma_start(y_r[:, c, :, :], ys_r[:, :, :])


def _prep_shards(x, Wq, lora_A, lora_B, Wk, Wv, Wo):
    bf16 = ml_dtypes.bfloat16
    fp8 = ml_dtypes.float8_e4m3
    f32 = np.float32

    xT = np.ascontiguousarray(x[0].T).astype(f32)
    xh = xT.astype(fp8)
    xl = (xT - xh.astype(f32)).astype(fp8)

    theta = 1.0 / (10000.0 ** (np.arange(0, HD, 2, dtype=np.float32) / HD))
    pos = np.arange(T, dtype=np.float32)
    ang = pos[:, None] * theta[None, :]
    ang = np.concatenate([ang, ang], axis=-1)          # [T, HD]
    cosT = np.cos(ang).T                               # [HD, T]
    sinT = np.sin(ang).T                               # unsigned; sign in perm
    cos2 = np.ascontiguousarray(np.concatenate([cosT, cosT], 0)).astype(bf16)
    sin2 = np.ascontiguousarray(np.concatenate([sinT, sinT], 0)).astype(bf16)

    # signed rotate-half permutation (per 64-row head block):
    # out[p] = sign(p) * src[rot(p)], rot = +-32 within the block
    perm = np.zeros((P, P), dtype=np.float32)
    for p in range(P):
        blk, q = (p // HD) * HD, p % HD
        rot = blk + (q + 32) % HD
        perm[rot, p] = -1.0 if q < 32 else 1.0
    perm = perm.astype(bf16)

    tri = (np.arange(P)[:, None] <= np.arange(P)[None, :]).astype(bf16)

    Wq_eff = Wq + lora_B.astype(np.float64) @ lora_A.astype(np.float64)
    Wq_eff = Wq_eff.astype(np.float32)

    in_maps = []
    for i in range(N_CORES):
        wq_i = Wq_eff[QW * i : QW * (i + 1), :]        # [256, D]
        wk_i = Wk[HD * i : HD * (i + 1), :]            # [64, D]
        wv_i = Wv[HD * i : HD * (i + 1), :]
        w_all = np.ascontiguousarray(
            np.concatenate([wq_i, wk_i, wv_i], 0).T * WS
        ).astype(f32)                                  # [D, 384]
        wh = w_all.astype(fp8)
        wl = (w_all - wh.astype(f32)).astype(fp8)
        # Wo^T rows for this core's head-dims: [256, D]
        woTf = np.ascontiguousarray(Wo[:, QW * i : QW * (i + 1)].T) * WS
        woT = (woTf / WS).astype(bf16)
        woh = woTf.astype(np.float32).astype(fp8)
        wol = (woTf.astype(np.float32) - woh.astype(np.float32)).astype(fp8)
        in_maps.append({
            "wo8h": woh,
            "wo8l": wol,
            "x8h": xh,
            "x8l": xl,
            "w8h": wh,
            "w8l": wl,
            "woT": woT,
            "cos2": cos2,
            "sin2": sin2,
            "tri": tri,
            "perm": perm,
        })
    return in_maps


def run(inputs, trace=False, **kw):
    nc = build_bass()
    in_maps = _prep_shards(**inputs)
    res = run_bass_kernel_spmd(
        nc, in_maps, core_ids=list(range(N_CORES)), trace=trace, **kw
    )
    # core i returns y^T rows [256*i, 256*(i+1)) = y columns, chunked over T
    cols = []
    for i in range(N_CORES):
        yc = np.asarray(res.results[i]["y"]).astype(np.float32)   # [NCH, 256, 512]
        cols.append(np.concatenate(list(yc), axis=1).T)           # [T, 256]
    return np.concatenate(cols, axis=1)[None], res


def kernel(**inputs):
    y, _ = run(inputs)
    return y


# revision 27
# speedup vs baseline: 1.0115x; 1.0115x over previous
"""GQA attention with LoRA-Q, tensor-parallel over 8 TRN2 cores.

Sharding (per core i of 8):
  - Q heads 4i..4i+3 (256 q-dims) and KV head i (GQA: repeat_interleave maps
    q heads [4i,4i+4) exactly onto kv head i).
  - Wq (with LoRA folded: Wq_eff = Wq + lora_B @ lora_A), Wk, Wv row-sharded;
    Wo row-parallel on its input (head) dim: each core computes the full-width
    partial y^T from its own 256 head-dims; four per-T-chunk ReduceScatter(add)
    ops (overlapped with attention of later chunks) leave each core its
    256-feature slice of y^T (transposed + concatenated on the host).

Structure is a single fused c-outer pipeline over the four 512-token chunks:
  proj c -> rope c -> attention (head pairs, shared kv) c -> norm c ->
  partial-Wo drain c -> ReduceScatter #c
so the collective and DMA traffic hide under attention of later chunks.

Precision: the QKV/V projections run as fp8e4 DoubleRow (256-deep contraction
per pass) in THREE split-precision terms — x and the (64x prescaled) weights
are each decomposed on the host into hi + lo fp8 parts and the x_lo*w_lo term
is dropped: xh@wh + xh@wl + xl@wh costs 3/4 of the bf16 chain at ~2e-3
relative error (better than bf16). Plain fp8 everywhere was measured at 2-5%
error per matmul family (relative error of a dot product equals the
per-element quantization error; it does not average down), which blows the
2e-2 budget, so scores/PV/Wo stay bf16. Softmax without max subtraction
(scores are bounded: |S/8| <= ~7), denominator fused into the PV matmul via
an appended ones-column on V. RoPE's rotate-half is a signed permutation
matmul on PE (no partition-shuffle DMAs). Score (ST) matmuls and the exp are
column-trimmed on the block-diagonal to skip fully-masked regions.
"""

import numpy as np
import ml_dtypes

import concourse.bass as bass
import concourse.mybir as mybir
import concourse.tile as tile
from concourse import bacc
from concourse.bass_utils import run_bass_kernel_spmd

BF16 = mybir.dt.bfloat16
F32 = mybir.dt.float32
FP8 = mybir.dt.float8e4
DR = mybir.MatmulPerfMode.DoubleRow

N_CORES = 8
T = 2048
D = 2048
HD = 64          # head dim
NH = 32          # total q heads
NKV = 8          # total kv heads
NH_LOC = NH // N_CORES       # 4 q heads per core
QW = NH_LOC * HD             # 256 q dims per core
P = 128
KT = D // P                  # 16 contraction tiles
KT2 = KT // 2                # 8 DoubleRow pairs
CH = 512                     # T-chunk
NCH = T // CH                # 4 chunks
NJ = T // P                  # 16 k-blocks
SCALE = 1.0 / 8.0            # 1/sqrt(64)
WS = 64.0                    # host-side weight scale (fp8 subnormal dodge)
IWS = 1.0 / WS


def build_bass():
    nc = bacc.Bacc(None, num_devices=N_CORES)

    # I/O
    xh_d = nc.dram_tensor("x8h", [D, T], FP8, kind="ExternalInput")
    xl_d = nc.dram_tensor("x8l", [D, T], FP8, kind="ExternalInput")
    wh_d = nc.dram_tensor("w8h", [D, QW + 2 * HD], FP8, kind="ExternalInput")
    wl_d = nc.dram_tensor("w8l", [D, QW + 2 * HD], FP8, kind="ExternalInput")
    woT_d = nc.dram_tensor("woT", [QW, D], BF16, kind="ExternalInput")
    woh_d = nc.dram_tensor("wo8h", [QW, D], FP8, kind="ExternalInput")
    wol_d = nc.dram_tensor("wo8l", [QW, D], FP8, kind="ExternalInput")
    cos2_d = nc.dram_tensor("cos2", [P, T], BF16, kind="ExternalInput")
    sin2_d = nc.dram_tensor("sin2", [P, T], BF16, kind="ExternalInput")
    tri_d = nc.dram_tensor("tri", [P, P], BF16, kind="ExternalInput")
    perm_d = nc.dram_tensor("perm", [P, P], BF16, kind="ExternalInput")
    y_d = nc.dram_tensor("y", [NCH, QW, CH], BF16, kind="ExternalOutput")

    with tile.TileContext(nc, num_cores=N_CORES) as tc:
        _body(nc, tc, xh_d, xl_d, wh_d, wl_d, woT_d, woh_d, wol_d, cos2_d,
              sin2_d, tri_d, perm_d, y_d)
    nc.compile()
    return nc


def _body(nc, tc, xh_d, xl_d, wh_d, wl_d, woT_d, woh_d, wol_d, cos2_d,
          sin2_d, tri_d, perm_d, y_d):
    import contextlib
    import itertools

    ctx = contextlib.ExitStack()
    with ctx:
        consts = ctx.enter_context(tc.tile_pool(name="consts", bufs=1))
        big = ctx.enter_context(tc.tile_pool(name="big", bufs=1))
        work = ctx.enter_context(tc.tile_pool(name="work", bufs=1))
        rp = ctx.enter_context(tc.tile_pool(name="rp", bufs=2))
        pt_p = ctx.enter_context(tc.tile_pool(name="pt_p", bufs=5))
        nrm = ctx.enter_context(tc.tile_pool(name="nrm", bufs=3))
        pst = ctx.enter_context(tc.tile_pool(name="pst", bufs=2))
        stP = ctx.enter_context(tc.tile_pool(name="stP", bufs=2, space="PSUM"))
        otP = ctx.enter_context(tc.tile_pool(name="otP", bufs=2, space="PSUM"))
        drP = ctx.enter_context(tc.tile_pool(name="drP", bufs=2, space="PSUM"))
        dram = ctx.enter_context(tc.tile_pool(name="dram", bufs=1, space="DRAM"))

        # ---- constant tiles (few big DMAs; the DMA path serializes per
        # dma_start, so count matters more than size)
        wh_sb = consts.tile([P, KT, QW + 2 * HD], FP8)
        wl_sb = consts.tile([P, KT, QW + 2 * HD], FP8)
        wh_r = wh_d.rearrange("(kt p) m -> p kt m", p=P)
        wl_r = wl_d.rearrange("(kt p) m -> p kt m", p=P)
        xh_sb = big.tile([P, KT, T], FP8, tag="bigh", name="xh_sb")
        xl_sb = big.tile([P, KT, T], FP8, tag="bigl", name="xl_sb")
        xh_r = xh_d.rearrange("(kt p) t -> p kt t", p=P)
        xl_r = xl_d.rearrange("(kt p) t -> p kt t", p=P)
        MK = 2 * P  # m=2 (k/v) weight columns start here; loaded first
        nc.sync.dma_start(wh_sb[:, :, MK:], wh_r[:, :, MK:])
        nc.sync.dma_start(xh_sb[:, :, 0:CH], xh_r[:, :, 0:CH])
        nc.sync.dma_start(wl_sb[:, :, MK:], wl_r[:, :, MK:])
        nc.sync.dma_start(xl_sb[:, :, 0:CH], xl_r[:, :, 0:CH])
        perm_sb = consts.tile([P, P], BF16)
        nc.sync.dma_start(perm_sb, perm_d[:])
        tri_sb = consts.tile([P, P], BF16)
        nc.sync.dma_start(tri_sb, tri_d[:])
        cos2_sb = consts.tile([P, T], BF16)
        nc.sync.dma_start(cos2_sb, cos2_d[:])
        sin2_sb = consts.tile([P, T], BF16)
        nc.sync.dma_start(sin2_sb, sin2_d[:])
        nc.sync.dma_start(wh_sb[:, :, 0:MK], wh_r[:, :, 0:MK])
        nc.sync.dma_start(wl_sb[:, :, 0:MK], wl_r[:, :, 0:MK])
        nc.sync.dma_start(xh_sb[:, :, CH : 2 * CH], xh_r[:, :, CH : 2 * CH])
        nc.sync.dma_start(xl_sb[:, :, CH : 2 * CH], xl_r[:, :, CH : 2 * CH])
        woT2_sb = consts.tile([P, 2, D], BF16)
        woT2_r = woT_d.rearrange("(g p) o -> p g o", p=P)
        woh_sb = consts.tile([P, 2, D], FP8)
        wol_sb = consts.tile([P, 2, D], FP8)
        nc.sync.dma_start(woh_sb, woh_d.rearrange("(g p) o -> p g o", p=P)[:, :, :])
        nc.sync.dma_start(wol_sb, wol_d.rearrange("(g p) o -> p g o", p=P)[:, :, :])

        ones64 = consts.tile([1, HD], BF16)
        nc.vector.memset(ones64, 1.0)

        # v with ones column appended: [tk(P), j, HD+1]
        v_aug = work.tile([P, NJ, HD + 1], BF16)
        nc.vector.memset(v_aug[:, :, HD : HD + 1], 1.0)

        # k^T duplicated on both partition halves so odd heads (whose rope
        # output lives at partitions 64:128) can matmul base-aligned
        kT2 = work.tile([P, T], BF16)
        # rope'd q pairs for all chunks: qAll[64*(h%2)+d, h//2, t]
        qAll = work.tile([P, 2, T], BF16)
        # O^T repacked to 128 partitions: OT128[64*(h%2)+d, h//2, t]
        # (partition p of pair g is local head-dim 128*g+p, matching woT2)
        OT128 = work.tile([P, 2, T], BF16)
        # final chunk's O in fp8 for the DoubleRow drain (the O-quantization
        # error only touches the last quarter of tokens: ~1.4%% overall)
        OT8 = work.tile([P, 2, CH], FP8)

        # per-chunk DRAM tiles: a shared tile would make chunk c+1's partT
        # writes falsely wait on ReduceScatter c's reads (coarse DRAM deps)
        partT_c = [
            dram.tile([D, CH], BF16, name=f"partT{c}", tag=f"partT{c}")
            for c in range(NCH)
        ]
        partT_rc = [t.rearrange("(ot p) t -> p ot t", p=P) for t in partT_c]
        ysT_c = [
            dram.tile([QW, CH], BF16, name=f"ysT{c}", tag=f"ysT{c}")
            for c in range(NCH)
        ]

        TERMS = None  # bound below; (w, x) split-precision term pairs

        def proj3_term(ps, wcols, rhs_sl, t, k2, rev=False):
            """One DoubleRow pair of split-precision term t (term-major so
            the hi*hi sweep starts after only the hi DMAs have landed)."""
            w, x = ((wh_sb, xh_sb), (wl_sb, xh_sb), (wh_sb, xl_sb))[t]
            if rev:
                lhsT = x[:, 2 * k2 : 2 * k2 + 2, rhs_sl]
                rhs = w[:, 2 * k2 : 2 * k2 + 2, wcols]
            else:
                lhsT = w[:, 2 * k2 : 2 * k2 + 2, wcols]
                rhs = x[:, 2 * k2 : 2 * k2 + 2, rhs_sl]
            nc.tensor.matmul(
                ps,
                lhsT=lhsT,
                rhs=rhs,
                start=(k2 == 0 and t == 0),
                stop=(k2 == KT2 - 1 and t == 2),
                perf_mode=DR,
            )

        def emit_proj_rope(c):
            """QKV projection + RoPE + t-partitioned V for chunk c, as a
            generator so chunks >= 1 can ride attention's PE bubbles."""
            sl = slice(c * CH, (c + 1) * CH)
            projT = rp.tile([P, 3, CH], BF16, tag="projT")
            for m in (2, 0, 1):
                ps = drP.tile([P, CH], F32, tag="dr")
                nmm = 0
                for t in range(3):
                    for k2 in range(KT2):
                        proj3_term(ps, slice(m * P, (m + 1) * P), sl, t, k2)
                        nmm += 1
                        if nmm % 8 == 7:
                            yield
                nc.vector.tensor_scalar_mul(projT[:, m, :], ps, IWS)
                yield
                if m == 2:
                    # RoPE k -> kT2[0:64, sl] + dup to [64:128]
                    ksh = drP.tile([P, CH], F32, tag="dr")
                    nc.tensor.matmul(
                        ksh[0:HD, :], lhsT=perm_sb[0:HD, 0:HD],
                        rhs=projT[0:HD, 2, :], start=True, stop=True,
                    )
                    kt2 = rp.tile([P, CH], BF16, tag="t2")
                    nc.vector.tensor_mul(kt2[0:HD, :], ksh[0:HD, :], sin2_sb[0:HD, sl])
                    veng = nc.vector if c == 0 else nc.gpsimd
                    kt1 = rp.tile([P, CH], BF16, tag="t1")
                    veng.tensor_mul(kt1[0:HD, :], projT[0:HD, 2, :], cos2_sb[0:HD, sl])
                    veng.tensor_add(kT2[0:HD, sl], kt1[0:HD, :], kt2[0:HD, :])
                    nc.gpsimd.dma_start(kT2[HD:P, sl], kT2[0:HD, sl])
                    if c == 1:
                        nc.sync.dma_start(woT2_sb, woT2_r[:, :, :])
                    if 1 <= c <= 2:
                        nx = slice((c + 1) * CH, (c + 2) * CH)
                        nc.sync.dma_start(xh_sb[:, :, nx], xh_r[:, :, nx])
                        nc.sync.dma_start(xl_sb[:, :, nx], xl_r[:, :, nx])
                    yield
                    # v in t-partitioned layout (tiny free dim: cheap matmuls)
                    for tb in range(4):
                        j = 4 * c + tb
                        pv = drP.tile([P, CH], F32, tag="dr")
                        for t in range(3):
                            for k2 in range(KT2):
                                proj3_term(
                                    pv[:, 0:HD],
                                    slice(QW + HD, QW + 2 * HD),
                                    slice(j * P, (j + 1) * P),
                                    t, k2, rev=True,
                                )
                        nc.vector.tensor_scalar_mul(
                            v_aug[:, j, 0:HD], pv[:, 0:HD], IWS
                        )
                        yield
                else:
                    s = m
                    qsh = drP.tile([P, CH], F32, tag="dr")
                    nc.tensor.matmul(
                        qsh, lhsT=perm_sb, rhs=projT[:, s, :],
                        start=True, stop=True,
                    )
                    yield
                    t2 = rp.tile([P, CH], BF16, tag="t2")
                    nc.vector.tensor_mul(t2, qsh, sin2_sb[:, sl])
                    veng = nc.vector if c == 0 else nc.gpsimd
                    t1 = rp.tile([P, CH], BF16, tag="t1")
                    veng.tensor_mul(t1, projT[:, s, :], cos2_sb[:, sl])
                    veng.tensor_add(qAll[:, s, sl], t1, t2)
                    yield

        def drain_gen(c, final=False):
            """Chunk c's partial-Wo drain + partT DMAs + ReduceScatter,
            interleaved into attention (c+1)'s j-loop as PE filler. The final
            drain feeds the tail ReduceScatter, so it balances the PSUM-drain
            copies across ACT+DVE and ships partT in small early pieces to
            get the collective started as soon as possible."""
            sl = slice(c * CH, (c + 1) * CH)
            fast = final or c == NCH - 2
            grp = 4
            pstage = pst.tile([P, KT, CH], BF16, tag="pst")
            for o_t in range(KT):
                # the final drain alternates between the drain and (by then
                # idle) attention PSUM pools: 4 banks of runway keep the
                # PSUM->SBUF copies fed so the scheduler never parks the
                # end-of-program drain barrier in front of them
                if final and o_t % 2 == 1:
                    ps = otP.tile([P, CH], F32, tag="ot")
                else:
                    ps = drP.tile([P, CH], F32, tag="dr")
                if final:
                    for t, w in enumerate((woh_sb, wol_sb)):
                        nc.tensor.matmul(
                            ps,
                            lhsT=w[:, :, o_t * P : (o_t + 1) * P],
                            rhs=OT8,
                            start=(t == 0),
                            stop=(t == 1),
                            perf_mode=DR,
                        )
                else:
                    for g in range(2):
                        nc.tensor.matmul(
                            ps,
                            lhsT=woT2_sb[:, g, o_t * P : (o_t + 1) * P],
                            rhs=OT128[:, g, sl],
                            start=(g == 0),
                            stop=(g == 1),
                        )
                yield
                if fast:
                    # late chunks feed the tail ReduceScatter chain: copies
                    # prefer DVE (ACT is exp-bound), with partT pieces issued
                    # from the gpsimd queue right after each odd copy so the
                    # collective starts the moment the drain lands.
                    if final and o_t % 2 == 0:
                        nc.scalar.mul(pstage[:, o_t, :], ps, IWS if final else 1.0)
                    elif final:
                        nc.vector.tensor_scalar_mul(pstage[:, o_t, :], ps, IWS)
                    else:
                        nc.vector.tensor_copy(pstage[:, o_t, :], ps)
                    if o_t % 4 == 3:
                        nc.gpsimd.dma_start(
                            partT_rc[c][:, o_t - 3 : o_t + 1, :],
                            pstage[:, o_t - 3 : o_t + 1, :],
                        )
                else:
                    if o_t % 4 == 0:
                        nc.scalar.copy(pstage[:, o_t, :], ps)
                    else:
                        nc.vector.tensor_copy(pstage[:, o_t, :], ps)
                    if o_t % grp == grp - 1:
                        hb = o_t - grp + 1
                        nc.sync.dma_start(
                            partT_rc[c][:, hb : o_t + 1, :],
                            pstage[:, hb : o_t + 1, :],
                        )
                yield
            emit_rs(c)

        def emit_rs(c):
            """ReduceScatter chunk c. The y output DMAs are all deferred to
            the end of the program: a y DMA waits ~21us on its collective,
            and on the in-order SP queue that wait would head-block the next
            chunk's partT DMAs, serializing every collective."""
            nc.gpsimd.collective_compute(
                "ReduceScatter",
                mybir.AluOpType.add,
                replica_groups=[list(range(N_CORES))],
                ins=[partT_c[c][:, :]],
                outs=[ysT_c[c][:, :]],
            )

        def emit_attention(c, filler, norm_now=False):
            """Attention for the 4 local heads of q-chunk c, exp grouped over
            2 adjacent k-blocks. Block-diagonal pairs are column-trimmed (the
            first 256*dg columns are fully masked). Pulls from filler between
            ST and PV so PE never idles while Exp runs."""
            nj2 = 2 * c + 2
            stg = nrm.tile([HD + 1, NH_LOC, CH], BF16, tag="stg")
            horder = (1, 3, 0, 2) if norm_now else (0, 2, 1, 3)
            for hpos, h in enumerate(horder):
                lo = HD * (h % 2)
                qrhs = qAll[lo : lo + HD, h // 2, c * CH : (c + 1) * CH]
                ot = otP.tile([P, CH], F32, tag="ot")

                def do_st(j2):
                    x0 = max(0, 2 * P * (j2 - 2 * c))
                    st = stP.tile([P, 2, CH], F32, tag="st")
                    for i in range(2):
                        nc.tensor.matmul(
                            st[:, i, x0:CH],
                            lhsT=kT2[lo : lo + HD, (2 * j2 + i) * P : (2 * j2 + i + 1) * P],
                            rhs=qrhs[:, x0:CH],
                            start=True,
                            stop=True,
                        )
                    return st

                EXPA = 128.0 * 1.4426950409 / 8.0   # bf16-bit exp slope
                EXPB = 16250.5                       # bias 16256 - sigma

                def do_rest(st, j2):
                    x0 = max(0, 2 * P * (j2 - 2 * c))
                    pt = pt_p.tile([P, 2, CH], BF16, tag="pt")
                    if False and c == NCH - 1 and hpos < 2 and j2 < 2 * c:
                        # ACT is the bottleneck of the last chunk: route a
                        # third of its off-diagonal exps to DVE as a
                        # Schraudolph bit-trick (synthesize the bf16 bit
                        # pattern of e^(s/8) with one mult+add, ~2% rms on
                        # ~12% of the softmax mass)
                        nc.vector.tensor_scalar(
                            pt.bitcast(mybir.dt.int16), st[:, :, :],
                            EXPA, EXPB,
                            mybir.AluOpType.mult, mybir.AluOpType.add,
                        )
                    else:
                        nc.scalar.activation(
                            pt[:, :, x0:CH], st[:, :, x0:CH],
                            mybir.ActivationFunctionType.Exp, scale=SCALE,
                        )
                    if j2 >= 2 * c:
                        # diagonal pair: per-block column trim. Block at diag
                        # offset d contributes nothing to cols < 128d; only
                        # the 128-wide triangle at [128d, 128d+128) needs
                        # masking; cols >= 128(d+1) are fully valid.
                        for i in range(2):
                            j = 2 * j2 + i
                            d = j - 4 * c
                            nc.vector.tensor_mul(
                                pt[:, i, P * d : P * (d + 1)],
                                pt[:, i, P * d : P * (d + 1)],
                                tri_sb,
                            )
                            nc.tensor.matmul(
                                ot[0 : HD + 1, P * d : CH],
                                lhsT=v_aug[:, j, :],
                                rhs=pt[:, i, P * d : CH],
                                start=(j == 0),
                                stop=(j == 4 * c + 3),
                                skip_group_check=True,
                            )
                    else:
                        for i in range(2):
                            j = 2 * j2 + i
                            nc.tensor.matmul(
                                ot[0 : HD + 1, :],
                                lhsT=v_aug[:, j, :],
                                rhs=pt[:, i, :],
                                start=(j == 0),
                                stop=False,
                                skip_group_check=True,
                            )

                st_cur = do_st(0)
                npull = 5 if c >= NCH - 2 else 3
                for j2 in range(nj2):
                    st_next = do_st(j2 + 1) if j2 + 1 < nj2 else None
                    next(filler, None)
                    do_rest(st_cur, j2)
                    for _ in range(npull - 1):
                        next(filler, None)
                    st_cur = st_next

                # stage unnormalized O^T + denominator row (bf16)
                if h % 2 == 0:
                    nc.scalar.copy(stg[:, h, :], ot[0 : HD + 1, :])
                else:
                    nc.vector.tensor_copy(stg[:, h, :], ot[0 : HD + 1, :])
                if norm_now:
                    # final chunk: normalize this head immediately so the
                    # partial-Wo drain can start right at attention end
                    for _ in norm_head(c, stg, h):
                        pass
            return stg

        def norm_head(c, stg, h):
            """Softmax normalization for one head of chunk c."""
            sl = slice(c * CH, (c + 1) * CH)
            rcpT = nrm.tile([1, NH_LOC, CH], BF16, tag="rcp")
            with nc.allow_low_precision("softmax denom in bf16 is fine"):
                nc.vector.reciprocal(
                    rcpT[0:1, h, :], stg[HD : HD + 1, h, :]
                )
            bc = otP.tile([P, CH], F32, tag="ot")
            nc.tensor.matmul(
                bc[0:HD, :], lhsT=ones64, rhs=rcpT[0:1, h, :],
                start=True, stop=True,
            )
            if c == NCH - 1:
                if h % 2 == 0:
                    nc.vector.tensor_mul(
                        OT8[0:HD, h // 2, :], stg[0:HD, h, :], bc[0:HD, :]
                    )
                else:
                    odd8 = nrm.tile([HD, CH], FP8, tag="odd8")
                    nc.vector.tensor_mul(odd8, stg[0:HD, h, :], bc[0:HD, :])
                    nc.gpsimd.dma_start(OT8[HD:P, h // 2, :], odd8)
            elif h % 2 == 0:
                nc.vector.tensor_mul(
                    OT128[0:HD, h // 2, sl], stg[0:HD, h, :], bc[0:HD, :]
                )
            else:
                oddt = nrm.tile([HD, CH], BF16, tag="oddt")
                nc.vector.tensor_mul(oddt, stg[0:HD, h, :], bc[0:HD, :])
                nc.gpsimd.dma_start(OT128[HD:P, h // 2, sl], oddt)
            yield

        def emit_norm(c, stg):
            """Softmax normalization for chunk c (4 heads)."""
            for h in range(NH_LOC):
                yield from norm_head(c, stg, h)

        # ---- prologue: chunk 0's proj/rope dense; everything else (proj of
        # later chunks, norm + partial-Wo drain + ReduceScatter of earlier
        # chunks) rides attention's PE bubbles via the filler generators.
        for _ in emit_proj_rope(0):
            pass
        filler = emit_proj_rope(1)
        for c in range(NCH):
            late = c >= NCH - 2
            stg = emit_attention(c, filler, norm_now=late)
            for _ in filler:
                pass
            gens = []
            if c + 2 < NCH:
                gens.append(emit_proj_rope(c + 2))
            if not late:
                gens.append(emit_norm(c, stg))
            gens.append(drain_gen(c, final=(c == NCH - 1)))
            filler = itertools.chain(*gens)
        # final norm+drain runs dense (its ReduceScatter is emitted inline)
        for _ in filler:
            pass
        y_r = y_d.rearrange("c (r p) t -> p c r t", p=P)
        for c in range(NCH):
            ys_r = ysT_c[c].rearrange("(r p) t -> p r t", p=P)
            nc.sync.dma_start(y_r[:, c, :, :], ys_r[:, :, :])


def _prep_shards(x, Wq, lora_A, lora_B, Wk, Wv, Wo):
    bf16 = ml_dtypes.bfloat16
    fp8 = ml_dtypes.float8_e4m3
    f32 = np.float32

    xT = np.ascontiguousarray(x[0].T).astype(f32)
    xh = xT.astype(fp8)
    xl = (xT - xh.astype(f32)).astype(fp8)

    theta = 1.0 / (10000.0 ** (np.arange(0, HD, 2, dtype=np.float32) / HD))
    pos = np.arange(T, dtype=np.float32)
    ang = pos[:, None] * theta[None, :]
    ang = np.concatenate([ang, ang], axis=-1)          # [T, HD]
    cosT = np.cos(ang).T                               # [HD, T]
    sinT = np.sin(ang).T                               # unsigned; sign in perm
    cos2 = np.ascontiguousarray(np.concatenate([cosT, cosT], 0)).astype(bf16)
    sin2 = np.ascontiguousarray(np.concatenate([sinT, sinT], 0)).astype(bf16)

    # signed rotate-half permutation (per 64-row head block):
    # out[p] = sign(p) * src[rot(p)], rot = +-32 within the block
    perm = np.zeros((P, P), dtype=np.float32)
    for p in range(P):
        blk, q = (p // HD) * HD, p % HD
        rot = blk + (q + 32) % HD
        perm[rot, p] = -1.0 if q < 32 else 1.0
    perm = perm.astype(bf16)

    tri = (np.arange(P)[:, None] <= np.arange(P)[None, :]).astype(bf16)

    Wq_eff = Wq + lora_B.astype(np.float64) @ lora_A.astype(np.float64)
    Wq_eff = Wq_eff.astype(np.float32)

    in_maps = []
    for i in range(N_CORES):
        wq_i = Wq_eff[QW * i : QW * (i + 1), :]        # [256, D]
        wk_i = Wk[HD * i : HD * (i + 1), :]            # [64, D]
        wv_i = Wv[HD * i : HD * (i + 1), :]
        w_all = np.ascontiguousarray(
            np.concatenate([wq_i, wk_i, wv_i], 0).T * WS
        ).astype(f32)                                  # [D, 384]
        wh = w_all.astype(fp8)
        wl = (w_all - wh.astype(f32)).astype(fp8)
        # Wo^T rows for this core's head-dims: [256, D]
        woTf = np.ascontiguousarray(Wo[:, QW * i : QW * (i + 1)].T) * WS
        woT = (woTf / WS).astype(bf16)
        woh = woTf.astype(np.float32).astype(fp8)
        wol = (woTf.astype(np.float32) - woh.astype(np.float32)).astype(fp8)
        in_maps.append({
            "wo8h": woh,
            "wo8l": wol,
            "x8h": xh,
            "x8l": xl,
            "w8h": wh,
            "w8l": wl,
            "woT": woT,
            "cos2": cos2,
            "sin2": sin2,
            "tri": tri,
            "perm": perm,
        })
    return in_maps


def run(inputs, trace=False, **kw):
    nc = build_bass()
    in_maps = _prep_shards(**inputs)
    res = run_bass_kernel_spmd(
        nc, in_maps, core_ids=list(range(N_CORES)), trace=trace, **kw
    )
    # core i returns y^T rows [256*i, 256*(i+1)) = y columns, chunked over T
    cols = []
    for i in range(N_CORES):
        yc = np.asarray(res.results[i]["y"]).astype(np.float32)   # [NCH, 256, 512]
        cols.append(np.concatenate(list(yc), axis=1).T)           # [T, 256]
    return np.concatenate(cols, axis=1)[None], res


def kernel(**inputs):
    y, _ = run(inputs)
    return y


# revision 28
# speedup vs baseline: 1.0119x; 1.0004x over previous
"""GQA attention with LoRA-Q, tensor-parallel over 8 TRN2 cores.

Sharding (per core i of 8):
  - Q heads 4i..4i+3 (256 q-dims) and KV head i (GQA: repeat_interleave maps
    q heads [4i,4i+4) exactly onto kv head i).
  - Wq (with LoRA folded: Wq_eff = Wq + lora_B @ lora_A), Wk, Wv row-sharded;
    Wo row-parallel on its input (head) dim: each core computes the full-width
    partial y^T from its own 256 head-dims; four per-T-chunk ReduceScatter(add)
    ops (overlapped with attention of later chunks) leave each core its
    256-feature slice of y^T (transposed + concatenated on the host).

Structure is a single fused c-outer pipeline over the four 512-token chunks:
  proj c -> rope c -> attention (head pairs, shared kv) c -> norm c ->
  partial-Wo drain c -> ReduceScatter #c
so the collective and DMA traffic hide under attention of later chunks.

Precision: the QKV/V projections run as fp8e4 DoubleRow (256-deep contraction
per pass) in THREE split-precision terms — x and the (64x prescaled) weights
are each decomposed on the host into hi + lo fp8 parts and the x_lo*w_lo term
is dropped: xh@wh + xh@wl + xl@wh costs 3/4 of the bf16 chain at ~2e-3
relative error (better than bf16). Plain fp8 everywhere was measured at 2-5%
error per matmul family (relative error of a dot product equals the
per-element quantization error; it does not average down), which blows the
2e-2 budget, so scores/PV/Wo stay bf16. Softmax without max subtraction
(scores are bounded: |S/8| <= ~7), denominator fused into the PV matmul via
an appended ones-column on V. RoPE's rotate-half is a signed permutation
matmul on PE (no partition-shuffle DMAs). Score (ST) matmuls and the exp are
column-trimmed on the block-diagonal to skip fully-masked regions.
"""

import numpy as np
import ml_dtypes

import concourse.bass as bass
import concourse.mybir as mybir
import concourse.tile as tile
from concourse import bacc
from concourse.bass_utils import run_bass_kernel_spmd

BF16 = mybir.dt.bfloat16
F32 = mybir.dt.float32
FP8 = mybir.dt.float8e4
DR = mybir.MatmulPerfMode.DoubleRow

N_CORES = 8
T = 2048
D = 2048
HD = 64          # head dim
NH = 32          # total q heads
NKV = 8          # total kv heads
NH_LOC = NH // N_CORES       # 4 q heads per core
QW = NH_LOC * HD             # 256 q dims per core
P = 128
KT = D // P                  # 16 contraction tiles
KT2 = KT // 2                # 8 DoubleRow pairs
CH = 512                     # T-chunk
NCH = T // CH                # 4 chunks
NJ = T // P                  # 16 k-blocks
SCALE = 1.0 / 8.0            # 1/sqrt(64)
WS = 64.0                    # host-side weight scale (fp8 subnormal dodge)
IWS = 1.0 / WS


def build_bass():
    nc = bacc.Bacc(None, num_devices=N_CORES)

    # I/O
    xh_d = nc.dram_tensor("x8h", [D, T], FP8, kind="ExternalInput")
    xl_d = nc.dram_tensor("x8l", [D, T], FP8, kind="ExternalInput")
    wh_d = nc.dram_tensor("w8h", [D, QW + 2 * HD], FP8, kind="ExternalInput")
    wl_d = nc.dram_tensor("w8l", [D, QW + 2 * HD], FP8, kind="ExternalInput")
    woT_d = nc.dram_tensor("woT", [QW, D], BF16, kind="ExternalInput")
    woh_d = nc.dram_tensor("wo8h", [QW, D], FP8, kind="ExternalInput")
    wol_d = nc.dram_tensor("wo8l", [QW, D], FP8, kind="ExternalInput")
    cos2_d = nc.dram_tensor("cos2", [P, T], BF16, kind="ExternalInput")
    sin2_d = nc.dram_tensor("sin2", [P, T], BF16, kind="ExternalInput")
    tri_d = nc.dram_tensor("tri", [P, P], BF16, kind="ExternalInput")
    perm_d = nc.dram_tensor("perm", [P, P], BF16, kind="ExternalInput")
    y_d = nc.dram_tensor("y", [NCH, QW, CH], BF16, kind="ExternalOutput")

    with tile.TileContext(nc, num_cores=N_CORES) as tc:
        _body(nc, tc, xh_d, xl_d, wh_d, wl_d, woT_d, woh_d, wol_d, cos2_d,
              sin2_d, tri_d, perm_d, y_d)
    nc.compile()
    return nc


def _body(nc, tc, xh_d, xl_d, wh_d, wl_d, woT_d, woh_d, wol_d, cos2_d,
          sin2_d, tri_d, perm_d, y_d):
    import contextlib
    import itertools

    ctx = contextlib.ExitStack()
    with ctx:
        consts = ctx.enter_context(tc.tile_pool(name="consts", bufs=1))
        big = ctx.enter_context(tc.tile_pool(name="big", bufs=1))
        work = ctx.enter_context(tc.tile_pool(name="work", bufs=1))
        rp = ctx.enter_context(tc.tile_pool(name="rp", bufs=2))
        pt_p = ctx.enter_context(tc.tile_pool(name="pt_p", bufs=6))
        nrm = ctx.enter_context(tc.tile_pool(name="nrm", bufs=3))
        pst = ctx.enter_context(tc.tile_pool(name="pst", bufs=2))
        stP = ctx.enter_context(tc.tile_pool(name="stP", bufs=2, space="PSUM"))
        otP = ctx.enter_context(tc.tile_pool(name="otP", bufs=2, space="PSUM"))
        drP = ctx.enter_context(tc.tile_pool(name="drP", bufs=2, space="PSUM"))
        dram = ctx.enter_context(tc.tile_pool(name="dram", bufs=1, space="DRAM"))

        # ---- constant tiles (few big DMAs; the DMA path serializes per
        # dma_start, so count matters more than size)
        wh_sb = consts.tile([P, KT, QW + 2 * HD], FP8)
        wl_sb = consts.tile([P, KT, QW + 2 * HD], FP8)
        wh_r = wh_d.rearrange("(kt p) m -> p kt m", p=P)
        wl_r = wl_d.rearrange("(kt p) m -> p kt m", p=P)
        xh_sb = big.tile([P, KT, T], FP8, tag="bigh", name="xh_sb")
        xl_sb = big.tile([P, KT, T], FP8, tag="bigl", name="xl_sb")
        xh_r = xh_d.rearrange("(kt p) t -> p kt t", p=P)
        xl_r = xl_d.rearrange("(kt p) t -> p kt t", p=P)
        MK = 2 * P  # m=2 (k/v) weight columns start here; loaded first
        nc.sync.dma_start(wh_sb[:, :, MK:], wh_r[:, :, MK:])
        nc.sync.dma_start(xh_sb[:, :, 0:CH], xh_r[:, :, 0:CH])
        nc.sync.dma_start(wl_sb[:, :, MK:], wl_r[:, :, MK:])
        nc.sync.dma_start(xl_sb[:, :, 0:CH], xl_r[:, :, 0:CH])
        perm_sb = consts.tile([P, P], BF16)
        nc.sync.dma_start(perm_sb, perm_d[:])
        tri_sb = consts.tile([P, P], BF16)
        nc.sync.dma_start(tri_sb, tri_d[:])
        cos2_sb = consts.tile([P, T], BF16)
        nc.sync.dma_start(cos2_sb, cos2_d[:])
        sin2_sb = consts.tile([P, T], BF16)
        nc.sync.dma_start(sin2_sb, sin2_d[:])
        nc.sync.dma_start(wh_sb[:, :, 0:MK], wh_r[:, :, 0:MK])
        nc.sync.dma_start(wl_sb[:, :, 0:MK], wl_r[:, :, 0:MK])
        nc.sync.dma_start(xh_sb[:, :, CH : 2 * CH], xh_r[:, :, CH : 2 * CH])
        nc.sync.dma_start(xl_sb[:, :, CH : 2 * CH], xl_r[:, :, CH : 2 * CH])
        woT2_sb = consts.tile([P, 2, D], BF16)
        woT2_r = woT_d.rearrange("(g p) o -> p g o", p=P)
        woh_sb = consts.tile([P, 2, D], FP8)
        wol_sb = consts.tile([P, 2, D], FP8)
        nc.sync.dma_start(woh_sb, woh_d.rearrange("(g p) o -> p g o", p=P)[:, :, :])
        nc.sync.dma_start(wol_sb, wol_d.rearrange("(g p) o -> p g o", p=P)[:, :, :])

        ones64 = consts.tile([1, HD], BF16)
        nc.vector.memset(ones64, 1.0)

        # v with ones column appended: [tk(P), j, HD+1]
        v_aug = work.tile([P, NJ, HD + 1], BF16)
        nc.vector.memset(v_aug[:, :, HD : HD + 1], 1.0)

        # k^T duplicated on both partition halves so odd heads (whose rope
        # output lives at partitions 64:128) can matmul base-aligned
        kT2 = work.tile([P, T], BF16)
        # rope'd q pairs for all chunks: qAll[64*(h%2)+d, h//2, t]
        qAll = work.tile([P, 2, T], BF16)
        # O^T repacked to 128 partitions: OT128[64*(h%2)+d, h//2, t]
        # (partition p of pair g is local head-dim 128*g+p, matching woT2)
        OT128 = work.tile([P, 2, T], BF16)
        # final chunk's O in fp8 for the DoubleRow drain (the O-quantization
        # error only touches the last quarter of tokens: ~1.4%% overall)
        OT8 = work.tile([P, 2, CH], FP8)

        # per-chunk DRAM tiles: a shared tile would make chunk c+1's partT
        # writes falsely wait on ReduceScatter c's reads (coarse DRAM deps)
        partT_c = [
            dram.tile([D, CH], BF16, name=f"partT{c}", tag=f"partT{c}")
            for c in range(NCH)
        ]
        partT_rc = [t.rearrange("(ot p) t -> p ot t", p=P) for t in partT_c]
        ysT_c = [
            dram.tile([QW, CH], BF16, name=f"ysT{c}", tag=f"ysT{c}")
            for c in range(NCH)
        ]

        TERMS = None  # bound below; (w, x) split-precision term pairs

        def proj3_term(ps, wcols, rhs_sl, t, k2, rev=False):
            """One DoubleRow pair of split-precision term t (term-major so
            the hi*hi sweep starts after only the hi DMAs have landed)."""
            w, x = ((wh_sb, xh_sb), (wl_sb, xh_sb), (wh_sb, xl_sb))[t]
            if rev:
                lhsT = x[:, 2 * k2 : 2 * k2 + 2, rhs_sl]
                rhs = w[:, 2 * k2 : 2 * k2 + 2, wcols]
            else:
                lhsT = w[:, 2 * k2 : 2 * k2 + 2, wcols]
                rhs = x[:, 2 * k2 : 2 * k2 + 2, rhs_sl]
            nc.tensor.matmul(
                ps,
                lhsT=lhsT,
                rhs=rhs,
                start=(k2 == 0 and t == 0),
                stop=(k2 == KT2 - 1 and t == 2),
                perf_mode=DR,
            )

        def emit_proj_rope(c):
            """QKV projection + RoPE + t-partitioned V for chunk c, as a
            generator so chunks >= 1 can ride attention's PE bubbles."""
            sl = slice(c * CH, (c + 1) * CH)
            projT = rp.tile([P, 3, CH], BF16, tag="projT")
            for m in (2, 0, 1):
                ps = drP.tile([P, CH], F32, tag="dr")
                nmm = 0
                for t in range(3):
                    for k2 in range(KT2):
                        proj3_term(ps, slice(m * P, (m + 1) * P), sl, t, k2)
                        nmm += 1
                        if nmm % 8 == 7:
                            yield
                nc.vector.tensor_scalar_mul(projT[:, m, :], ps, IWS)
                yield
                if m == 2:
                    # RoPE k -> kT2[0:64, sl] + dup to [64:128]
                    ksh = drP.tile([P, CH], F32, tag="dr")
                    nc.tensor.matmul(
                        ksh[0:HD, :], lhsT=perm_sb[0:HD, 0:HD],
                        rhs=projT[0:HD, 2, :], start=True, stop=True,
                    )
                    kt2 = rp.tile([P, CH], BF16, tag="t2")
                    nc.vector.tensor_mul(kt2[0:HD, :], ksh[0:HD, :], sin2_sb[0:HD, sl])
                    veng = nc.vector if c == 0 else nc.gpsimd
                    kt1 = rp.tile([P, CH], BF16, tag="t1")
                    veng.tensor_mul(kt1[0:HD, :], projT[0:HD, 2, :], cos2_sb[0:HD, sl])
                    veng.tensor_add(kT2[0:HD, sl], kt1[0:HD, :], kt2[0:HD, :])
                    nc.gpsimd.dma_start(kT2[HD:P, sl], kT2[0:HD, sl])
                    if c == 1:
                        nc.sync.dma_start(woT2_sb, woT2_r[:, :, :])
                    if 1 <= c <= 2:
                        nx = slice((c + 1) * CH, (c + 2) * CH)
                        nc.sync.dma_start(xh_sb[:, :, nx], xh_r[:, :, nx])
                        nc.sync.dma_start(xl_sb[:, :, nx], xl_r[:, :, nx])
                    yield
                    # v in t-partitioned layout (tiny free dim: cheap matmuls)
                    for tb in range(4):
                        j = 4 * c + tb
                        pv = drP.tile([P, CH], F32, tag="dr")
                        for t in range(3):
                            for k2 in range(KT2):
                                proj3_term(
                                    pv[:, 0:HD],
                                    slice(QW + HD, QW + 2 * HD),
                                    slice(j * P, (j + 1) * P),
                                    t, k2, rev=True,
                                )
                        nc.vector.tensor_scalar_mul(
                            v_aug[:, j, 0:HD], pv[:, 0:HD], IWS
                        )
                        yield
                else:
                    s = m
                    qsh = drP.tile([P, CH], F32, tag="dr")
                    nc.tensor.matmul(
                        qsh, lhsT=perm_sb, rhs=projT[:, s, :],
                        start=True, stop=True,
                    )
                    yield
                    t2 = rp.tile([P, CH], BF16, tag="t2")
                    nc.vector.tensor_mul(t2, qsh, sin2_sb[:, sl])
                    veng = nc.vector if c == 0 else nc.gpsimd
                    t1 = rp.tile([P, CH], BF16, tag="t1")
                    veng.tensor_mul(t1, projT[:, s, :], cos2_sb[:, sl])
                    veng.tensor_add(qAll[:, s, sl], t1, t2)
                    yield

        def drain_gen(c, final=False):
            """Chunk c's partial-Wo drain + partT DMAs + ReduceScatter,
            interleaved into attention (c+1)'s j-loop as PE filler. The final
            drain feeds the tail ReduceScatter, so it balances the PSUM-drain
            copies across ACT+DVE and ships partT in small early pieces to
            get the collective started as soon as possible."""
            sl = slice(c * CH, (c + 1) * CH)
            fast = final or c == NCH - 2
            grp = 4
            pstage = pst.tile([P, KT, CH], BF16, tag="pst")
            for o_t in range(KT):
                # the final drain alternates between the drain and (by then
                # idle) attention PSUM pools: 4 banks of runway keep the
                # PSUM->SBUF copies fed so the scheduler never parks the
                # end-of-program drain barrier in front of them
                if final and o_t % 2 == 1:
                    ps = otP.tile([P, CH], F32, tag="ot")
                else:
                    ps = drP.tile([P, CH], F32, tag="dr")
                if final:
                    for t, w in enumerate((woh_sb, wol_sb)):
                        nc.tensor.matmul(
                            ps,
                            lhsT=w[:, :, o_t * P : (o_t + 1) * P],
                            rhs=OT8,
                            start=(t == 0),
                            stop=(t == 1),
                            perf_mode=DR,
                        )
                else:
                    for g in range(2):
                        nc.tensor.matmul(
                            ps,
                            lhsT=woT2_sb[:, g, o_t * P : (o_t + 1) * P],
                            rhs=OT128[:, g, sl],
                            start=(g == 0),
                            stop=(g == 1),
                        )
                yield
                if fast:
                    # late chunks feed the tail ReduceScatter chain: copies
                    # prefer DVE (ACT is exp-bound), with partT pieces issued
                    # from the gpsimd queue right after each odd copy so the
                    # collective starts the moment the drain lands.
                    if final and o_t % 2 == 0:
                        nc.scalar.mul(pstage[:, o_t, :], ps, IWS if final else 1.0)
                    elif final:
                        nc.vector.tensor_scalar_mul(pstage[:, o_t, :], ps, IWS)
                    else:
                        nc.vector.tensor_copy(pstage[:, o_t, :], ps)
                    if o_t % 4 == 3:
                        nc.gpsimd.dma_start(
                            partT_rc[c][:, o_t - 3 : o_t + 1, :],
                            pstage[:, o_t - 3 : o_t + 1, :],
                        )
                else:
                    if o_t % 4 == 0:
                        nc.scalar.copy(pstage[:, o_t, :], ps)
                    else:
                        nc.vector.tensor_copy(pstage[:, o_t, :], ps)
                    if o_t % grp == grp - 1:
                        hb = o_t - grp + 1
                        nc.sync.dma_start(
                            partT_rc[c][:, hb : o_t + 1, :],
                            pstage[:, hb : o_t + 1, :],
                        )
                yield
            emit_rs(c)

        def emit_rs(c):
            """ReduceScatter chunk c. The y output DMAs are all deferred to
            the end of the program: a y DMA waits ~21us on its collective,
            and on the in-order SP queue that wait would head-block the next
            chunk's partT DMAs, serializing every collective."""
            nc.gpsimd.collective_compute(
                "ReduceScatter",
                mybir.AluOpType.add,
                replica_groups=[list(range(N_CORES))],
                ins=[partT_c[c][:, :]],
                outs=[ysT_c[c][:, :]],
            )

        def emit_attention(c, filler, norm_now=False):
            """Attention for the 4 local heads of q-chunk c, exp grouped over
            2 adjacent k-blocks. Block-diagonal pairs are column-trimmed (the
            first 256*dg columns are fully masked). Pulls from filler between
            ST and PV so PE never idles while Exp runs."""
            nj2 = 2 * c + 2
            stg = nrm.tile([HD + 1, NH_LOC, CH], BF16, tag="stg")
            horder = (1, 3, 0, 2) if norm_now else (0, 2, 1, 3)
            for hpos, h in enumerate(horder):
                lo = HD * (h % 2)
                qrhs = qAll[lo : lo + HD, h // 2, c * CH : (c + 1) * CH]
                ot = otP.tile([P, CH], F32, tag="ot")

                def do_st(j2):
                    x0 = max(0, 2 * P * (j2 - 2 * c))
                    st = stP.tile([P, 2, CH], F32, tag="st")
                    for i in range(2):
                        nc.tensor.matmul(
                            st[:, i, x0:CH],
                            lhsT=kT2[lo : lo + HD, (2 * j2 + i) * P : (2 * j2 + i + 1) * P],
                            rhs=qrhs[:, x0:CH],
                            start=True,
                            stop=True,
                        )
                    return st

                EXPA = 128.0 * 1.4426950409 / 8.0   # bf16-bit exp slope
                EXPB = 16250.5                       # bias 16256 - sigma

                def do_rest(st, j2):
                    x0 = max(0, 2 * P * (j2 - 2 * c))
                    pt = pt_p.tile([P, 2, CH], BF16, tag="pt")
                    if False and c == NCH - 1 and hpos < 2 and j2 < 2 * c:
                        # ACT is the bottleneck of the last chunk: route a
                        # third of its off-diagonal exps to DVE as a
                        # Schraudolph bit-trick (synthesize the bf16 bit
                        # pattern of e^(s/8) with one mult+add, ~2% rms on
                        # ~12% of the softmax mass)
                        nc.vector.tensor_scalar(
                            pt.bitcast(mybir.dt.int16), st[:, :, :],
                            EXPA, EXPB,
                            mybir.AluOpType.mult, mybir.AluOpType.add,
                        )
                    else:
                        nc.scalar.activation(
                            pt[:, :, x0:CH], st[:, :, x0:CH],
                            mybir.ActivationFunctionType.Exp, scale=SCALE,
                        )
                    if j2 >= 2 * c:
                        # diagonal pair: per-block column trim. Block at diag
                        # offset d contributes nothing to cols < 128d; only
                        # the 128-wide triangle at [128d, 128d+128) needs
                        # masking; cols >= 128(d+1) are fully valid.
                        for i in range(2):
                            j = 2 * j2 + i
                            d = j - 4 * c
                            nc.vector.tensor_mul(
                                pt[:, i, P * d : P * (d + 1)],
                                pt[:, i, P * d : P * (d + 1)],
                                tri_sb,
                            )
                            nc.tensor.matmul(
                                ot[0 : HD + 1, P * d : CH],
                                lhsT=v_aug[:, j, :],
                                rhs=pt[:, i, P * d : CH],
                                start=(j == 0),
                                stop=(j == 4 * c + 3),
                                skip_group_check=True,
                            )
                    else:
                        for i in range(2):
                            j = 2 * j2 + i
                            nc.tensor.matmul(
                                ot[0 : HD + 1, :],
                                lhsT=v_aug[:, j, :],
                                rhs=pt[:, i, :],
                                start=(j == 0),
                                stop=False,
                                skip_group_check=True,
                            )

                st_cur = do_st(0)
                npull = 5 if c >= NCH - 2 else 3
                for j2 in range(nj2):
                    st_next = do_st(j2 + 1) if j2 + 1 < nj2 else None
                    next(filler, None)
                    do_rest(st_cur, j2)
                    for _ in range(npull - 1):
                        next(filler, None)
                    st_cur = st_next

                # stage unnormalized O^T + denominator row (bf16)
                if h % 2 == 0:
                    nc.scalar.copy(stg[:, h, :], ot[0 : HD + 1, :])
                else:
                    nc.vector.tensor_copy(stg[:, h, :], ot[0 : HD + 1, :])
                if norm_now:
                    # final chunk: normalize this head immediately so the
                    # partial-Wo drain can start right at attention end
                    for _ in norm_head(c, stg, h):
                        pass
            return stg

        def norm_head(c, stg, h):
            """Softmax normalization for one head of chunk c."""
            sl = slice(c * CH, (c + 1) * CH)
            rcpT = nrm.tile([1, NH_LOC, CH], BF16, tag="rcp")
            with nc.allow_low_precision("softmax denom in bf16 is fine"):
                nc.vector.reciprocal(
                    rcpT[0:1, h, :], stg[HD : HD + 1, h, :]
                )
            bc = otP.tile([P, CH], F32, tag="ot")
            nc.tensor.matmul(
                bc[0:HD, :], lhsT=ones64, rhs=rcpT[0:1, h, :],
                start=True, stop=True,
            )
            if c == NCH - 1:
                if h % 2 == 0:
                    nc.vector.tensor_mul(
                        OT8[0:HD, h // 2, :], stg[0:HD, h, :], bc[0:HD, :]
                    )
                else:
                    odd8 = nrm.tile([HD, CH], FP8, tag="odd8")
                    nc.vector.tensor_mul(odd8, stg[0:HD, h, :], bc[0:HD, :])
                    nc.gpsimd.dma_start(OT8[HD:P, h // 2, :], odd8)
            elif h % 2 == 0:
                nc.vector.tensor_mul(
                    OT128[0:HD, h // 2, sl], stg[0:HD, h, :], bc[0:HD, :]
                )
            else:
                oddt = nrm.tile([HD, CH], BF16, tag="oddt")
                nc.vector.tensor_mul(oddt, stg[0:HD, h, :], bc[0:HD, :])
                nc.gpsimd.dma_start(OT128[HD:P, h // 2, sl], oddt)
            yield

        def emit_norm(c, stg):
            """Softmax normalization for chunk c (4 heads)."""
            for h in range(NH_LOC):
                yield from norm_head(c, stg, h)

        # ---- prologue: chunk 0's proj/rope dense; everything else (proj of
        # later chunks, norm + partial-Wo drain + ReduceScatter of earlier
        # chunks) rides attention's PE bubbles via the filler generators.
        for _ in emit_proj_rope(0):
            pass
        filler = emit_proj_rope(1)
        for c in range(NCH):
            late = c >= NCH - 2
            stg = emit_attention(c, filler, norm_now=late)
            for _ in filler:
                pass
            gens = []
            if c + 2 < NCH:
                gens.append(emit_proj_rope(c + 2))
            if not late:
                gens.append(emit_norm(c, stg))
            gens.append(drain_gen(c, final=(c == NCH - 1)))
            filler = itertools.chain(*gens)
        # final norm+drain runs dense (its ReduceScatter is emitted inline)
        for _ in filler:
            pass
        y_r = y_d.rearrange("c (r p) t -> p c r t", p=P)
        for c in range(NCH):
            ys_r = ysT_c[c].rearrange("(r p) t -> p r t", p=P)
            nc.sync.dma_start(y_r[:, c, :, :], ys_r[:, :, :])


def _prep_shards(x, Wq, lora_A, lora_B, Wk, Wv, Wo):
    bf16 = ml_dtypes.bfloat16
    fp8 = ml_dtypes.float8_e4m3
    f32 = np.float32

    xT = np.ascontiguousarray(x[0].T).astype(f32)
    xh = xT.astype(fp8)
    xl = (xT - xh.astype(f32)).astype(fp8)

    theta = 1.0 / (10000.0 ** (np.arange(0, HD, 2, dtype=np.float32) / HD))
    pos = np.arange(T, dtype=np.float32)
    ang = pos[:, None] * theta[None, :]
    ang = np.concatenate([ang, ang], axis=-1)          # [T, HD]
    cosT = np.cos(ang).T                               # [HD, T]
    sinT = np.sin(ang).T                               # unsigned; sign in perm
    cos2 = np.ascontiguousarray(np.concatenate([cosT, cosT], 0)).astype(bf16)
    sin2 = np.ascontiguousarray(np.concatenate([sinT, sinT], 0)).astype(bf16)

    # signed rotate-half permutation (per 64-row head block):
    # out[p] = sign(p) * src[rot(p)], rot = +-32 within the block
    perm = np.zeros((P, P), dtype=np.float32)
    for p in range(P):
        blk, q = (p // HD) * HD, p % HD
        rot = blk + (q + 32) % HD
        perm[rot, p] = -1.0 if q < 32 else 1.0
    perm = perm.astype(bf16)

    tri = (np.arange(P)[:, None] <= np.arange(P)[None, :]).astype(bf16)

    Wq_eff = Wq + lora_B.astype(np.float64) @ lora_A.astype(np.float64)
    Wq_eff = Wq_eff.astype(np.float32)

    in_maps = []
    for i in range(N_CORES):
        wq_i = Wq_eff[QW * i : QW * (i + 1), :]        # [256, D]
        wk_i = Wk[HD * i : HD * (i + 1), :]            # [64, D]
        wv_i = Wv[HD * i : HD * (i + 1), :]
        w_all = np.ascontiguousarray(
            np.concatenate([wq_i, wk_i, wv_i], 0).T * WS
        ).astype(f32)                                  # [D, 384]
        wh = w_all.astype(fp8)
        wl = (w_all - wh.astype(f32)).astype(fp8)
        # Wo^T rows for this core's head-dims: [256, D]
        woTf = np.ascontiguousarray(Wo[:, QW * i : QW * (i + 1)].T) * WS
        woT = (woTf / WS).astype(bf16)
        woh = woTf.astype(np.float32).astype(fp8)
        wol = (woTf.astype(np.float32) - woh.astype(np.float32)).astype(fp8)
        in_maps.append({
            "wo8h": woh,
            "wo8l": wol,
            "x8h": xh,
            "x8l": xl,
            "w8h": wh,
            "w8l": wl,
            "woT": woT,
            "cos2": cos2,
            "sin2": sin2,
            "tri": tri,
            "perm": perm,
        })
    return in_maps


def run(inputs, trace=False, **kw):
    nc = build_bass()
    in_maps = _prep_shards(**inputs)
    res = run_bass_kernel_spmd(
        nc, in_maps, core_ids=list(range(N_CORES)), trace=trace, **kw
    )
    # core i returns y^T rows [256*i, 256*(i+1)) = y columns, chunked over T
    cols = []
    for i in range(N_CORES):
        yc = np.asarray(res.results[i]["y"]).astype(np.float32)   # [NCH, 256, 512]
        cols.append(np.concatenate(list(yc), axis=1).T)           # [T, 256]
    return np.concatenate(cols, axis=1)[None], res


def kernel(**inputs):
    y, _ = run(inputs)
    return y


# revision 29
# speedup vs baseline: 1.0253x; 1.0132x over previous
"""GQA attention with LoRA-Q, tensor-parallel over 8 TRN2 cores.

Sharding (per core i of 8):
  - Q heads 4i..4i+3 (256 q-dims) and KV head i (GQA: repeat_interleave maps
    q heads [4i,4i+4) exactly onto kv head i).
  - Wq (with LoRA folded: Wq_eff = Wq + lora_B @ lora_A), Wk, Wv row-sharded;
    Wo row-parallel on its input (head) dim: each core computes the full-width
    partial y^T from its own 256 head-dims; four per-T-chunk ReduceScatter(add)
    ops (overlapped with attention of later chunks) leave each core its
    256-feature slice of y^T (transposed + concatenated on the host).

Structure is a single fused c-outer pipeline over the four 512-token chunks:
  proj c -> rope c -> attention (head pairs, shared kv) c -> norm c ->
  partial-Wo drain c -> ReduceScatter #c
so the collective and DMA traffic hide under attention of later chunks.

Precision: the QKV/V projections run as fp8e4 DoubleRow (256-deep contraction
per pass) in THREE split-precision terms — x and the (64x prescaled) weights
are each decomposed on the host into hi + lo fp8 parts and the x_lo*w_lo term
is dropped: xh@wh + xh@wl + xl@wh costs 3/4 of the bf16 chain at ~2e-3
relative error (better than bf16). Plain fp8 everywhere was measured at 2-5%
error per matmul family (relative error of a dot product equals the
per-element quantization error; it does not average down), which blows the
2e-2 budget, so scores/PV/Wo stay bf16. Softmax without max subtraction
(scores are bounded: |S/8| <= ~7), denominator fused into the PV matmul via
an appended ones-column on V. RoPE's rotate-half is a signed permutation
matmul on PE (no partition-shuffle DMAs). Score (ST) matmuls and the exp are
column-trimmed on the block-diagonal to skip fully-masked regions.
"""

import numpy as np
import ml_dtypes

import concourse.bass as bass
import concourse.mybir as mybir
import concourse.tile as tile
from concourse import bacc
from concourse.bass_utils import run_bass_kernel_spmd

BF16 = mybir.dt.bfloat16
F32 = mybir.dt.float32
FP8 = mybir.dt.float8e4
DR = mybir.MatmulPerfMode.DoubleRow

N_CORES = 8
T = 2048
D = 2048
HD = 64          # head dim
NH = 32          # total q heads
NKV = 8          # total kv heads
NH_LOC = NH // N_CORES       # 4 q heads per core
QW = NH_LOC * HD             # 256 q dims per core
P = 128
KT = D // P                  # 16 contraction tiles
KT2 = KT // 2                # 8 DoubleRow pairs
CH = 512                     # T-chunk
NCH = T // CH                # 4 chunks
NJ = T // P                  # 16 k-blocks
SCALE = 1.0 / 8.0            # 1/sqrt(64)
WS = 64.0                    # host-side weight scale (fp8 subnormal dodge)
IWS = 1.0 / WS


def build_bass():
    nc = bacc.Bacc(None, num_devices=N_CORES)

    # I/O
    xh_d = nc.dram_tensor("x8h", [D, T], FP8, kind="ExternalInput")
    xl_d = nc.dram_tensor("x8l", [D, T], FP8, kind="ExternalInput")
    wh_d = nc.dram_tensor("w8h", [D, QW + 2 * HD], FP8, kind="ExternalInput")
    wl_d = nc.dram_tensor("w8l", [D, QW + 2 * HD], FP8, kind="ExternalInput")
    woT_d = nc.dram_tensor("woT", [QW, D], BF16, kind="ExternalInput")
    woh_d = nc.dram_tensor("wo8h", [QW, D], FP8, kind="ExternalInput")
    wol_d = nc.dram_tensor("wo8l", [QW, D], FP8, kind="ExternalInput")
    cos2_d = nc.dram_tensor("cos2", [P, T], BF16, kind="ExternalInput")
    sin2_d = nc.dram_tensor("sin2", [P, T], BF16, kind="ExternalInput")
    tri_d = nc.dram_tensor("tri", [P, P], BF16, kind="ExternalInput")
    perm_d = nc.dram_tensor("perm", [P, P], BF16, kind="ExternalInput")
    y_d = nc.dram_tensor("y", [NCH, QW, CH], BF16, kind="ExternalOutput")

    with tile.TileContext(nc, num_cores=N_CORES) as tc:
        _body(nc, tc, xh_d, xl_d, wh_d, wl_d, woT_d, woh_d, wol_d, cos2_d,
              sin2_d, tri_d, perm_d, y_d)
    nc.compile()
    return nc


def _body(nc, tc, xh_d, xl_d, wh_d, wl_d, woT_d, woh_d, wol_d, cos2_d,
          sin2_d, tri_d, perm_d, y_d):
    import contextlib
    import itertools

    ctx = contextlib.ExitStack()
    with ctx:
        consts = ctx.enter_context(tc.tile_pool(name="consts", bufs=1))
        big = ctx.enter_context(tc.tile_pool(name="big", bufs=1))
        work = ctx.enter_context(tc.tile_pool(name="work", bufs=1))
        rp = ctx.enter_context(tc.tile_pool(name="rp", bufs=2))
        pt_p = ctx.enter_context(tc.tile_pool(name="pt_p", bufs=6))
        nrm = ctx.enter_context(tc.tile_pool(name="nrm", bufs=3))
        pst = ctx.enter_context(tc.tile_pool(name="pst", bufs=2))
        stP = ctx.enter_context(tc.tile_pool(name="stP", bufs=2, space="PSUM"))
        otP = ctx.enter_context(tc.tile_pool(name="otP", bufs=2, space="PSUM"))
        drP = ctx.enter_context(tc.tile_pool(name="drP", bufs=2, space="PSUM"))
        dram = ctx.enter_context(tc.tile_pool(name="dram", bufs=1, space="DRAM"))

        # ---- constant tiles (few big DMAs; the DMA path serializes per
        # dma_start, so count matters more than size)
        wh_sb = consts.tile([P, KT, QW + 2 * HD], FP8)
        wl_sb = consts.tile([P, KT, QW + 2 * HD], FP8)
        wh_r = wh_d.rearrange("(kt p) m -> p kt m", p=P)
        wl_r = wl_d.rearrange("(kt p) m -> p kt m", p=P)
        xh_sb = big.tile([P, KT, T], FP8, tag="bigh", name="xh_sb")
        xl_sb = big.tile([P, KT, T], FP8, tag="bigl", name="xl_sb")
        xh_r = xh_d.rearrange("(kt p) t -> p kt t", p=P)
        xl_r = xl_d.rearrange("(kt p) t -> p kt t", p=P)
        MK = 2 * P  # m=2 (k/v) weight columns start here; loaded first
        nc.sync.dma_start(wh_sb[:, :, MK:], wh_r[:, :, MK:])
        nc.sync.dma_start(xh_sb[:, :, 0:CH], xh_r[:, :, 0:CH])
        nc.sync.dma_start(wl_sb[:, :, MK:], wl_r[:, :, MK:])
        nc.sync.dma_start(xl_sb[:, :, 0:CH], xl_r[:, :, 0:CH])
        perm_sb = consts.tile([P, P], BF16)
        nc.sync.dma_start(perm_sb, perm_d[:])
        tri_sb = consts.tile([P, P], BF16)
        nc.sync.dma_start(tri_sb, tri_d[:])
        cos2_sb = consts.tile([P, T], BF16)
        nc.sync.dma_start(cos2_sb, cos2_d[:])
        sin2_sb = consts.tile([P, T], BF16)
        nc.sync.dma_start(sin2_sb, sin2_d[:])
        nc.sync.dma_start(wh_sb[:, :, 0:MK], wh_r[:, :, 0:MK])
        nc.sync.dma_start(wl_sb[:, :, 0:MK], wl_r[:, :, 0:MK])
        nc.sync.dma_start(xh_sb[:, :, CH : 2 * CH], xh_r[:, :, CH : 2 * CH])
        nc.sync.dma_start(xl_sb[:, :, CH : 2 * CH], xl_r[:, :, CH : 2 * CH])
        woT2_sb = consts.tile([P, 2, D], BF16)
        woT2_r = woT_d.rearrange("(g p) o -> p g o", p=P)
        woh_sb = consts.tile([P, 2, D], FP8)
        wol_sb = consts.tile([P, 2, D], FP8)
        nc.sync.dma_start(woh_sb, woh_d.rearrange("(g p) o -> p g o", p=P)[:, :, :])
        nc.sync.dma_start(wol_sb, wol_d.rearrange("(g p) o -> p g o", p=P)[:, :, :])

        ones64 = consts.tile([1, HD], BF16)
        nc.vector.memset(ones64, 1.0)

        # v with ones column appended: [tk(P), j, HD+1]
        v_aug = work.tile([P, NJ, HD + 1], BF16)
        nc.vector.memset(v_aug[:, :, HD : HD + 1], 1.0)

        # k^T duplicated on both partition halves so odd heads (whose rope
        # output lives at partitions 64:128) can matmul base-aligned
        kT2 = work.tile([P, T], BF16)
        # rope'd q pairs for all chunks: qAll[64*(h%2)+d, h//2, t]
        qAll = work.tile([P, 2, T], BF16)
        # O^T repacked to 128 partitions: OT128[64*(h%2)+d, h//2, t]
        # (partition p of pair g is local head-dim 128*g+p, matching woT2)
        OT128 = work.tile([P, 2, T], BF16)
        # final chunk's O in fp8 for the DoubleRow drain (the O-quantization
        # error only touches the last quarter of tokens: ~1.4%% overall)
        OT8 = work.tile([P, 2, CH], FP8)

        # per-chunk DRAM tiles: a shared tile would make chunk c+1's partT
        # writes falsely wait on ReduceScatter c's reads (coarse DRAM deps)
        partT_c = [
            dram.tile([D, CH], BF16, name=f"partT{c}", tag=f"partT{c}")
            for c in range(NCH)
        ]
        partT_rc = [t.rearrange("(ot p) t -> p ot t", p=P) for t in partT_c]
        ysT_c = [
            dram.tile([QW, CH], BF16, name=f"ysT{c}", tag=f"ysT{c}")
            for c in range(NCH)
        ]

        TERMS = None  # bound below; (w, x) split-precision term pairs

        def proj3_term(ps, wcols, rhs_sl, t, k2, rev=False):
            """One DoubleRow pair of split-precision term t (term-major so
            the hi*hi sweep starts after only the hi DMAs have landed)."""
            w, x = ((wh_sb, xh_sb), (wl_sb, xh_sb), (wh_sb, xl_sb))[t]
            if rev:
                lhsT = x[:, 2 * k2 : 2 * k2 + 2, rhs_sl]
                rhs = w[:, 2 * k2 : 2 * k2 + 2, wcols]
            else:
                lhsT = w[:, 2 * k2 : 2 * k2 + 2, wcols]
                rhs = x[:, 2 * k2 : 2 * k2 + 2, rhs_sl]
            nc.tensor.matmul(
                ps,
                lhsT=lhsT,
                rhs=rhs,
                start=(k2 == 0 and t == 0),
                stop=(k2 == KT2 - 1 and t == 2),
                perf_mode=DR,
            )

        def emit_proj_rope(c):
            """QKV projection + RoPE + t-partitioned V for chunk c, as a
            generator so chunks >= 1 can ride attention's PE bubbles."""
            sl = slice(c * CH, (c + 1) * CH)
            projT = rp.tile([P, 3, CH], BF16, tag="projT")
            for m in (2, 0, 1):
                ps = drP.tile([P, CH], F32, tag="dr")
                nmm = 0
                for t in range(3):
                    for k2 in range(KT2):
                        proj3_term(ps, slice(m * P, (m + 1) * P), sl, t, k2)
                        nmm += 1
                        if nmm % 8 == 7:
                            yield
                nc.vector.tensor_scalar_mul(projT[:, m, :], ps, IWS)
                yield
                if m == 2:
                    # RoPE k -> kT2[0:64, sl] + dup to [64:128]
                    ksh = drP.tile([P, CH], F32, tag="dr")
                    nc.tensor.matmul(
                        ksh[0:HD, :], lhsT=perm_sb[0:HD, 0:HD],
                        rhs=projT[0:HD, 2, :], start=True, stop=True,
                    )
                    kt2 = rp.tile([P, CH], BF16, tag="t2")
                    nc.vector.tensor_mul(kt2[0:HD, :], ksh[0:HD, :], sin2_sb[0:HD, sl])
                    veng = nc.vector if c == 0 else nc.gpsimd
                    kt1 = rp.tile([P, CH], BF16, tag="t1")
                    veng.tensor_mul(kt1[0:HD, :], projT[0:HD, 2, :], cos2_sb[0:HD, sl])
                    veng.tensor_add(kT2[0:HD, sl], kt1[0:HD, :], kt2[0:HD, :])
                    nc.gpsimd.dma_start(kT2[HD:P, sl], kT2[0:HD, sl])
                    if c == 1:
                        nc.sync.dma_start(woT2_sb, woT2_r[:, :, :])
                    if 1 <= c <= 2:
                        nx = slice((c + 1) * CH, (c + 2) * CH)
                        nc.sync.dma_start(xh_sb[:, :, nx], xh_r[:, :, nx])
                        nc.sync.dma_start(xl_sb[:, :, nx], xl_r[:, :, nx])
                    yield
                    # v in t-partitioned layout (tiny free dim: cheap matmuls)
                    for tb in range(4):
                        j = 4 * c + tb
                        pv = drP.tile([P, CH], F32, tag="dr")
                        for t in range(3):
                            for k2 in range(KT2):
                                proj3_term(
                                    pv[:, 0:HD],
                                    slice(QW + HD, QW + 2 * HD),
                                    slice(j * P, (j + 1) * P),
                                    t, k2, rev=True,
                                )
                        nc.vector.tensor_scalar_mul(
                            v_aug[:, j, 0:HD], pv[:, 0:HD], IWS
                        )
                        yield
                else:
                    s = m
                    qsh = drP.tile([P, CH], F32, tag="dr")
                    nc.tensor.matmul(
                        qsh, lhsT=perm_sb, rhs=projT[:, s, :],
                        start=True, stop=True,
                    )
                    yield
                    t2 = rp.tile([P, CH], BF16, tag="t2")
                    nc.vector.tensor_mul(t2, qsh, sin2_sb[:, sl])
                    veng = nc.vector if c == 0 else nc.gpsimd
                    t1 = rp.tile([P, CH], BF16, tag="t1")
                    veng.tensor_mul(t1, projT[:, s, :], cos2_sb[:, sl])
                    veng.tensor_add(qAll[:, s, sl], t1, t2)
                    yield

        def drain_gen(c, final=False):
            """Chunk c's partial-Wo drain + partT DMAs + ReduceScatter,
            interleaved into attention (c+1)'s j-loop as PE filler. The final
            drain feeds the tail ReduceScatter, so it balances the PSUM-drain
            copies across ACT+DVE and ships partT in small early pieces to
            get the collective started as soon as possible."""
            sl = slice(c * CH, (c + 1) * CH)
            fast = final or c == NCH - 2
            grp = 4
            pstage = pst.tile([P, KT, CH], BF16, tag="pst")
            for o_t in range(KT):
                # the final drain alternates between the drain and (by then
                # idle) attention PSUM pools: 4 banks of runway keep the
                # PSUM->SBUF copies fed so the scheduler never parks the
                # end-of-program drain barrier in front of them
                if final and o_t % 2 == 1:
                    ps = otP.tile([P, CH], F32, tag="ot")
                else:
                    ps = drP.tile([P, CH], F32, tag="dr")
                if final:
                    for t, w in enumerate((woh_sb, wol_sb)):
                        nc.tensor.matmul(
                            ps,
                            lhsT=w[:, :, o_t * P : (o_t + 1) * P],
                            rhs=OT8,
                            start=(t == 0),
                            stop=(t == 1),
                            perf_mode=DR,
                        )
                else:
                    for g in range(2):
                        nc.tensor.matmul(
                            ps,
                            lhsT=woT2_sb[:, g, o_t * P : (o_t + 1) * P],
                            rhs=OT128[:, g, sl],
                            start=(g == 0),
                            stop=(g == 1),
                        )
                yield
                if fast:
                    # late chunks feed the tail ReduceScatter chain: copies
                    # prefer DVE (ACT is exp-bound), with partT pieces issued
                    # from the gpsimd queue right after each odd copy so the
                    # collective starts the moment the drain lands.
                    if final and o_t % 2 == 0:
                        nc.scalar.mul(pstage[:, o_t, :], ps, IWS if final else 1.0)
                    elif final:
                        nc.vector.tensor_scalar_mul(pstage[:, o_t, :], ps, IWS)
                    else:
                        nc.vector.tensor_copy(pstage[:, o_t, :], ps)
                    if o_t % 4 == 3:
                        nc.gpsimd.dma_start(
                            partT_rc[c][:, o_t - 3 : o_t + 1, :],
                            pstage[:, o_t - 3 : o_t + 1, :],
                        )
                else:
                    if o_t % 4 == 0:
                        nc.scalar.copy(pstage[:, o_t, :], ps)
                    else:
                        nc.vector.tensor_copy(pstage[:, o_t, :], ps)
                    if o_t % grp == grp - 1:
                        hb = o_t - grp + 1
                        nc.sync.dma_start(
                            partT_rc[c][:, hb : o_t + 1, :],
                            pstage[:, hb : o_t + 1, :],
                        )
                yield
            emit_rs(c)

        def emit_rs(c):
            """ReduceScatter chunk c. The y output DMAs are all deferred to
            the end of the program: a y DMA waits ~21us on its collective,
            and on the in-order SP queue that wait would head-block the next
            chunk's partT DMAs, serializing every collective."""
            nc.gpsimd.collective_compute(
                "ReduceScatter",
                mybir.AluOpType.add,
                replica_groups=[list(range(N_CORES))],
                ins=[partT_c[c][:, :]],
                outs=[ysT_c[c][:, :]],
            )

        def emit_attention(c, filler, norm_now=False):
            """Attention for the 4 local heads of q-chunk c, exp grouped over
            2 adjacent k-blocks. Block-diagonal pairs are column-trimmed (the
            first 256*dg columns are fully masked). Pulls from filler between
            ST and PV so PE never idles while Exp runs."""
            nj2 = 2 * c + 2
            stg = nrm.tile([HD + 1, NH_LOC, CH], BF16, tag="stg")
            horder = (1, 3, 0, 2) if norm_now else (0, 2, 1, 3)
            for hpos, h in enumerate(horder):
                lo = HD * (h % 2)
                qrhs = qAll[lo : lo + HD, h // 2, c * CH : (c + 1) * CH]
                ot = otP.tile([P, CH], F32, tag="ot")

                def do_st(j2):
                    x0 = max(0, 2 * P * (j2 - 2 * c))
                    st = stP.tile([P, 2, CH], F32, tag="st")
                    for i in range(2):
                        nc.tensor.matmul(
                            st[:, i, x0:CH],
                            lhsT=kT2[lo : lo + HD, (2 * j2 + i) * P : (2 * j2 + i + 1) * P],
                            rhs=qrhs[:, x0:CH],
                            start=True,
                            stop=True,
                        )
                    return st

                EXPA = 128.0 * 1.4426950409 / 8.0   # bf16-bit exp slope
                EXPB = 16250.5                       # bias 16256 - sigma

                def do_rest(st, j2):
                    x0 = max(0, 2 * P * (j2 - 2 * c))
                    pt = pt_p.tile([P, 2, CH], BF16, tag="pt")
                    if False and c == NCH - 1 and hpos < 2 and j2 < 2 * c:
                        # ACT is the bottleneck of the last chunk: route a
                        # third of its off-diagonal exps to DVE as a
                        # Schraudolph bit-trick (synthesize the bf16 bit
                        # pattern of e^(s/8) with one mult+add, ~2% rms on
                        # ~12% of the softmax mass)
                        nc.vector.tensor_scalar(
                            pt.bitcast(mybir.dt.int16), st[:, :, :],
                            EXPA, EXPB,
                            mybir.AluOpType.mult, mybir.AluOpType.add,
                        )
                    else:
                        nc.scalar.activation(
                            pt[:, :, x0:CH], st[:, :, x0:CH],
                            mybir.ActivationFunctionType.Exp, scale=SCALE,
                        )
                    if j2 >= 2 * c:
                        # diagonal pair: per-block column trim. Block at diag
                        # offset d contributes nothing to cols < 128d; only
                        # the 128-wide triangle at [128d, 128d+128) needs
                        # masking; cols >= 128(d+1) are fully valid.
                        for i in range(2):
                            j = 2 * j2 + i
                            d = j - 4 * c
                            nc.vector.tensor_mul(
                                pt[:, i, P * d : P * (d + 1)],
                                pt[:, i, P * d : P * (d + 1)],
                                tri_sb,
                            )
                            nc.tensor.matmul(
                                ot[0 : HD + 1, P * d : CH],
                                lhsT=v_aug[:, j, :],
                                rhs=pt[:, i, P * d : CH],
                                start=(j == 0),
                                stop=(j == 4 * c + 3),
                                skip_group_check=True,
                            )
                    else:
                        for i in range(2):
                            j = 2 * j2 + i
                            nc.tensor.matmul(
                                ot[0 : HD + 1, :],
                                lhsT=v_aug[:, j, :],
                                rhs=pt[:, i, :],
                                start=(j == 0),
                                stop=False,
                                skip_group_check=True,
                            )

                st_cur = do_st(0)
                npull = 5 if c >= NCH - 2 else 3
                for j2 in range(nj2):
                    st_next = do_st(j2 + 1) if j2 + 1 < nj2 else None
                    next(filler, None)
                    do_rest(st_cur, j2)
                    for _ in range(npull - 1):
                        next(filler, None)
                    st_cur = st_next

                # stage unnormalized O^T + denominator row (bf16)
                if h % 2 == 0:
                    nc.scalar.copy(stg[:, h, :], ot[0 : HD + 1, :])
                else:
                    nc.vector.tensor_copy(stg[:, h, :], ot[0 : HD + 1, :])
                if norm_now:
                    # final chunk: normalize this head immediately so the
                    # partial-Wo drain can start right at attention end
                    for _ in norm_head(c, stg, h):
                        pass
            return stg

        def norm_head(c, stg, h):
            """Softmax normalization for one head of chunk c."""
            sl = slice(c * CH, (c + 1) * CH)
            rcpT = nrm.tile([1, NH_LOC, CH], BF16, tag="rcp")
            with nc.allow_low_precision("softmax denom in bf16 is fine"):
                nc.vector.reciprocal(
                    rcpT[0:1, h, :], stg[HD : HD + 1, h, :]
                )
            bc = drP.tile([P, CH], F32, tag="dr")
            nc.tensor.matmul(
                bc[0:HD, :], lhsT=ones64, rhs=rcpT[0:1, h, :],
                start=True, stop=True,
            )
            if c == NCH - 1:
                if h % 2 == 0:
                    nc.vector.tensor_mul(
                        OT8[0:HD, h // 2, :], stg[0:HD, h, :], bc[0:HD, :]
                    )
                else:
                    odd8 = nrm.tile([HD, CH], FP8, tag="odd8")
                    nc.vector.tensor_mul(odd8, stg[0:HD, h, :], bc[0:HD, :])
                    nc.gpsimd.dma_start(OT8[HD:P, h // 2, :], odd8)
            elif h % 2 == 0:
                nc.vector.tensor_mul(
                    OT128[0:HD, h // 2, sl], stg[0:HD, h, :], bc[0:HD, :]
                )
            else:
                oddt = nrm.tile([HD, CH], BF16, tag="oddt")
                nc.vector.tensor_mul(oddt, stg[0:HD, h, :], bc[0:HD, :])
                nc.gpsimd.dma_start(OT128[HD:P, h // 2, sl], oddt)
            yield

        def emit_norm(c, stg):
            """Softmax normalization for chunk c (4 heads)."""
            for h in range(NH_LOC):
                yield from norm_head(c, stg, h)

        # ---- prologue: chunk 0's proj/rope dense; everything else (proj of
        # later chunks, norm + partial-Wo drain + ReduceScatter of earlier
        # chunks) rides attention's PE bubbles via the filler generators.
        for _ in emit_proj_rope(0):
            pass
        filler = emit_proj_rope(1)
        for c in range(NCH):
            late = c >= NCH - 2
            stg = emit_attention(c, filler, norm_now=late)
            for _ in filler:
                pass
            gens = []
            if c + 2 < NCH:
                gens.append(emit_proj_rope(c + 2))
            if not late:
                gens.append(emit_norm(c, stg))
            gens.append(drain_gen(c, final=(c == NCH - 1)))
            filler = itertools.chain(*gens)
        # final norm+drain runs dense (its ReduceScatter is emitted inline)
        for _ in filler:
            pass
        y_r = y_d.rearrange("c (r p) t -> p c r t", p=P)
        for c in range(NCH):
            ys_r = ysT_c[c].rearrange("(r p) t -> p r t", p=P)
            nc.sync.dma_start(y_r[:, c, :, :], ys_r[:, :, :])


def _prep_shards(x, Wq, lora_A, lora_B, Wk, Wv, Wo):
    bf16 = ml_dtypes.bfloat16
    fp8 = ml_dtypes.float8_e4m3
    f32 = np.float32

    xT = np.ascontiguousarray(x[0].T).astype(f32)
    xh = xT.astype(fp8)
    xl = (xT - xh.astype(f32)).astype(fp8)

    theta = 1.0 / (10000.0 ** (np.arange(0, HD, 2, dtype=np.float32) / HD))
    pos = np.arange(T, dtype=np.float32)
    ang = pos[:, None] * theta[None, :]
    ang = np.concatenate([ang, ang], axis=-1)          # [T, HD]
    cosT = np.cos(ang).T                               # [HD, T]
    sinT = np.sin(ang).T                               # unsigned; sign in perm
    cos2 = np.ascontiguousarray(np.concatenate([cosT, cosT], 0)).astype(bf16)
    sin2 = np.ascontiguousarray(np.concatenate([sinT, sinT], 0)).astype(bf16)

    # signed rotate-half permutation (per 64-row head block):
    # out[p] = sign(p) * src[rot(p)], rot = +-32 within the block
    perm = np.zeros((P, P), dtype=np.float32)
    for p in range(P):
        blk, q = (p // HD) * HD, p % HD
        rot = blk + (q + 32) % HD
        perm[rot, p] = -1.0 if q < 32 else 1.0
    perm = perm.astype(bf16)

    tri = (np.arange(P)[:, None] <= np.arange(P)[None, :]).astype(bf16)

    Wq_eff = Wq + lora_B.astype(np.float64) @ lora_A.astype(np.float64)
    Wq_eff = Wq_eff.astype(np.float32)

    in_maps = []
    for i in range(N_CORES):
        wq_i = Wq_eff[QW * i : QW * (i + 1), :]        # [256, D]
        wk_i = Wk[HD * i : HD * (i + 1), :]            # [64, D]
        wv_i = Wv[HD * i : HD * (i + 1), :]
        w_all = np.ascontiguousarray(
            np.concatenate([wq_i, wk_i, wv_i], 0).T * WS
        ).astype(f32)                                  # [D, 384]
        wh = w_all.astype(fp8)
        wl = (w_all - wh.astype(f32)).astype(fp8)
        # Wo^T rows for this core's head-dims: [256, D]
        woTf = np.ascontiguousarray(Wo[:, QW * i : QW * (i + 1)].T) * WS
        woT = (woTf / WS).astype(bf16)
        woh = woTf.astype(np.float32).astype(fp8)
        wol = (woTf.astype(np.float32) - woh.astype(np.float32)).astype(fp8)
        in_maps.append({
            "wo8h": woh,
            "wo8l": wol,
            "x8h": xh,
            "x8l": xl,
            "w8h": wh,
            "w8l": wl,
            "woT": woT,
            "cos2": cos2,
            "sin2": sin2,
            "tri": tri,
            "perm": perm,
        })
    return in_maps


def run(inputs, trace=False, **kw):
    nc = build_bass()
    in_maps = _prep_shards(**inputs)
    res = run_bass_kernel_spmd(
        nc, in_maps, core_ids=list(range(N_CORES)), trace=trace, **kw
    )
    # core i returns y^T rows [256*i, 256*(i+1)) = y columns, chunked over T
    cols = []
    for i in range(N_CORES):
        yc = np.asarray(res.results[i]["y"]).astype(np.float32)   # [NCH, 256, 512]
        cols.append(np.concatenate(list(yc), axis=1).T)           # [T, 256]
    return np.concatenate(cols, axis=1)[None], res


def kernel(**inputs):
    y, _ = run(inputs)
    return y
